# revision 1
# baseline (speedup 1.0000x reference)
# Trainium2 Bass kernel for residual-VQ autoencoder loss (vq_codebook).
# Data-parallel over rows: 8 NeuronCores, 2048 rows each; codebooks/weights
# replicated. The scalar loss is assembled on host from per-core [128,40]
# partial-sum outputs (no on-chip collectives needed).
#
# Per core (RT=16 row-tiles of 128):
#   encoder: h = x@W1+b1 -> LN -> ReLU -> latent = h@W2' + b2 (ln_g folded into W2)
#   RVQ: psum_score[r,v] = 2K * r . E_v  (bf16 matmul, pre-scaled E^T)
#     one custom-DVE pass per [128,2048] PSUM group adds K*(SHIFT-|E_v|^2),
#     quantizes via (x+BIG)-BIG to multiples of 8192, adds the global vocab
#     index, and MAX-accumulates (chained across 4 groups) => packed argmax.
#     idx = packed mod 8192; q = codebook[idx] via gpsimd.dma_gather (exact f32);
#     resid -= q^T (PE transpose + DVE sub).
#   loss telescopes: sum_l 1.5*mean((q_l-r_l)^2) = 1.5*(R0-R4)/(N*LAT);
#   quant^T = latent^T - resid^T; decoder stays feature-major (no transposes);
#   Rrec = sum((recon-x)^2) using x^T spilled to DRAM during the encoder.

import sys

sys.path.insert(0, "/opt/trn_rl_repo")

import numpy as np

import concourse.bass as bass
import concourse.mybir as mybir
import concourse.bacc as bacc
import concourse.tile as tile
from concourse.alu_op_type import AluOpType
from concourse.bass_utils import run_bass_kernel_spmd

OBS, HID, LAT = 1024, 2048, 256
VOCAB, HQ = 8192, 4
N, NCORES = 16384, 8
NSH = N // NCORES          # 2048 rows per core
RT = NSH // 128            # 16 row tiles
LN_EPS = 1e-5
GRID = 2048.0               # one vocab group per QPACK call; 11 index bits
BIG = float(1.5 * 2.0**34)  # ulp(BIG)=2048 -> (x+BIG)-BIG rounds to mult of GRID

f32 = mybir.dt.float32
f32r = mybir.dt.float32r
bf16 = mybir.dt.bfloat16
i16 = mybir.dt.int16

_QPACK = None


def _register_qpack():
    """out = (((Src0+Src1)+BIG)-BIG) + Idx; accum_out = max over free dim.
    Src0: PSUM scores 2K*r.E (f32); Src1: K*(SHIFT-|E|^2) replicated (bf16).
    Quantizes the score to multiples of GRID and packs the within-group index
    into the low bits; per-group accums are combined at level end."""
    global _QPACK
    if _QPACK is not None:
        return _QPACK
    from concourse import dve_ops
    from concourse.dve_spec import Spec, Src0, Src1, C2, AluOp, Idx, lower
    from concourse.dve_table_gen import DveOpSpec

    def _ref(in0, in1, s0, s1, imm2):
        x = np.asarray(in0, np.float32) + np.asarray(in1, np.float32)
        q = (x + np.float32(imm2)).astype(np.float32) - np.float32(imm2)
        idxv = np.arange(x.shape[-1], dtype=np.float32)
        out = (q + idxv).astype(np.float32)
        acc = out.max(axis=-1, keepdims=True)
        return out, acc

    body = (((Src0 + Src1) + C2) - C2) + Idx
    spec = Spec(body=body, accum=AluOp.MAX, reference=_ref)
    op = dve_ops.DveOp("QPACK_ARGMAX", spec, subdim=False, uops_sha={})
    dve_ops.OPS.append(op)
    dve_ops.CUSTOM_DVE_SPECS[op.name] = op.spec
    dve_ops._SUB_OPCODE_FOR_NAME[op.name] = (
        dve_ops._CUSTOM_DVE_ROW_BASE + len(dve_ops.OPS) - 1
    )
    for ver in ("v3", "v4"):
        s = DveOpSpec(
            name=op.name,
            opcode=dve_ops.get_dve_sub_opcode(op.name),
            uops=lower(spec, ver=ver),
            rd1_en=True,
        )
        op.uops_sha[ver] = s.sha(ver)
    _QPACK = op
    return op


def build_nc(use_b1=True, use_b2=True, use_db1=True, use_db2=True):
    qpack = _register_qpack()
    nc = bacc.Bacc(
        "TRN2",
        target_bir_lowering=False,
        debug=False,
        enable_asserts=False,
        num_devices=NCORES,
    )
    Relu = mybir.ActivationFunctionType.Relu
    Square = mybir.ActivationFunctionType.Square
    Sqrt = mybir.ActivationFunctionType.Sqrt

    # ---------------- DRAM I/O ----------------
    x_d = nc.dram_tensor("xbt", [OBS, NSH], bf16, kind="ExternalInput")
    w1_d = nc.dram_tensor("w1b", [OBS, HID], bf16, kind="ExternalInput")
    b1_d = nc.dram_tensor("b1", [HID // 512, 512], bf16, kind="ExternalInput")
    w2_d = nc.dram_tensor("w2b", [HID, LAT], bf16, kind="ExternalInput")
    b2_d = nc.dram_tensor("b2", [1, LAT], bf16, kind="ExternalInput")
    dw1_d = nc.dram_tensor("dw1b", [LAT, HID], bf16, kind="ExternalInput")
    db1_d = nc.dram_tensor("db1", [HID // 512, 512], bf16, kind="ExternalInput")
    dw2_d = nc.dram_tensor("dw2b", [HID, OBS], bf16, kind="ExternalInput")
    db2_d = nc.dram_tensor("db2", [OBS // 512, 512], bf16, kind="ExternalInput")
    e2t_d = nc.dram_tensor("e2t", [HQ, LAT, VOCAB], bf16, kind="ExternalInput")
    se2_d = nc.dram_tensor("se2", [HQ, 128, VOCAB], bf16, kind="ExternalInput")
    e2tp_d = nc.dram_tensor("e2tp", [HQ, 128, VOCAB], mybir.dt.uint32, kind="ExternalInput")
    identb_d = nc.dram_tensor("identb", [128, 128], bf16, kind="ExternalInput")
    d_giota = nc.dram_tensor("giota", [128, RT * 4], f32, kind="ExternalInput")
    out_d = nc.dram_tensor("out", [128, 40], f32, kind="ExternalOutput")

    import contextlib

    with tile.TileContext(nc) as tc, contextlib.ExitStack() as ctx:
        const_p = ctx.enter_context(tc.tile_pool(name="const", bufs=1))
        persist_p = ctx.enter_context(tc.tile_pool(name="persist", bufs=1))
        small_p = ctx.enter_context(tc.tile_pool(name="small", bufs=4))

        # ---- constants ----
        identb = const_p.tile([128, 128], bf16, name="identb")
        nc.sync.dma_start(identb[:], identb_d.ap())
        ones1 = const_p.tile([1, 512], bf16, name="ones1")
        nc.vector.memset(ones1[:], 1.0)
        out_sb = const_p.tile([128, 40], f32, name="out_sb")
        nc.vector.memset(out_sb[:], 0.0)
        epsc = const_p.tile([128, 1], f32, name="epsc")
        nc.vector.memset(epsc[:], LN_EPS)

        # ---- persistent ----
        latT = persist_p.tile([128, 2, NSH], f32, name="latT")
        residT = persist_p.tile([128, 2, NSH], f32, name="residT")
        residTb = persist_p.tile([128, 2, NSH], bf16, name="residTb")
        trash = persist_p.tile([128, 2048], bf16, name="trash")
        idx16 = persist_p.tile([128, RT], i16, name="idx16")
        idxg = persist_p.tile([128, RT, 8], i16, name="idxg")
        nc.vector.memset(idxg[:], 0)
        # =============== encoder ===============
        enc_ctx = contextlib.ExitStack()
        enc_p = enc_ctx.enter_context(tc.tile_pool(name="encp", bufs=1))
        work_p = enc_ctx.enter_context(tc.tile_pool(name="encw", bufs=2))
        eps_h = enc_ctx.enter_context(tc.tile_pool(name="epsh", bufs=6, space="PSUM"))
        eps_t = enc_ctx.enter_context(tc.tile_pool(name="epst", bufs=2, space="PSUM"))
        w2s = enc_p.tile([128, HID // 128, LAT], bf16, name="w2s")
        w1s = enc_p.tile([128, OBS // 128, HID], bf16, name="w1s")
        for k in range(OBS // 128):
            nc.sync.dma_start(w1s[:, k, :], w1_d.ap()[k * 128:(k + 1) * 128, :])
        for k in range(HID // 128):
            nc.sync.dma_start(w2s[:, k, :], w2_d.ap()[k * 128:(k + 1) * 128, :])
        b1s = const_p.tile([HID // 512, 512], bf16, name="b1s")
        if use_b1:
            nc.sync.dma_start(b1s[:], b1_d.ap())
        b2s = const_p.tile([1, LAT], bf16, name="b2s")
        if use_b2:
            nc.sync.dma_start(b2s[:], b2_d.ap())

        xT_sb = enc_p.tile([128, OBS // 128, NSH], bf16, name="xT_sb")
        for k in range(OBS // 128):
            nc.sync.dma_start(xT_sb[:, k, :], x_d.ap()[k * 128:(k + 1) * 128, :])
        for rc in range(8):  # row chunks of 256 (2 row tiles each)
            hTc = work_p.tile([128, HID // 128, 256], bf16, name="hTc", tag="hT")
            for rj in range(2):
                rt = rc * 2 + rj
                # h = x @ W1 (+ b1), chunk-grained PSUM for overlap
                hsb = work_p.tile([128, 2048], f32, name="hsb", tag="hsb")
                bns = small_p.tile([128, 4, 6], f32, name="bns", tag="s1")
                for cc in range(4):
                    hps = eps_h.tile([128, 512], f32, name="hps", tag="hps")
                    nmm = OBS // 128
                    for k in range(nmm):
                        nc.tensor.matmul(
                            hps[:],
                            xT_sb[:, k, rt * 128:(rt + 1) * 128],
                            w1s[:, k, cc * 512:(cc + 1) * 512],
                            start=(k == 0),
                            stop=(k == nmm - 1 and not use_b1),
                        )
                    if use_b1:
                        nc.tensor.matmul(
                            hps[:],
                            ones1[:, 0:128],
                            b1s[cc:cc + 1, :],
                            start=False, stop=True,
                        )
                    nc.scalar.activation(
                        hsb[:, cc * 512:(cc + 1) * 512], hps[:],
                        mybir.ActivationFunctionType.Copy,
                    )
                    nc.vector.bn_stats(bns[:, cc, :], hps[:])
                mv = small_p.tile([128, 2], f32, name="mv", tag="s2")
                nc.vector.bn_aggr(mv[:], bns[:].rearrange("p a b -> p (a b)"))
                std = small_p.tile([128, 1], f32, name="std", tag="s6")
                nc.scalar.activation(std[:], mv[:, 1:2], Sqrt, bias=epsc[:])
                rstd = small_p.tile([128, 1], f32, name="rstd", tag="s7")
                nc.vector.reciprocal(rstd[:], std[:])
                nmr = small_p.tile([128, 1], f32, name="nmr", tag="s8")
                nc.vector.tensor_scalar(
                    nmr[:], mv[:, 0:1], rstd[:], -1.0,
                    op0=AluOpType.mult, op1=AluOpType.mult,
                )
                hrelu = work_p.tile([128, 2048], bf16, name="hrelu", tag="hrelu")
                nc.scalar.activation(
                    hrelu[:], hsb[:], Relu, bias=nmr[:], scale=rstd[:]
                )
                for o in range(HID // 128):
                    htp = eps_t.tile([128, 128], f32, name="htp", tag="tp")
                    nc.tensor.matmul(
                        htp[:, 0:64].bitcast(bf16),
                        hrelu[:, o * 128:(o + 1) * 128],
                        identb[:], is_transpose=True, start=True, stop=True,
                    )
                    nc.vector.tensor_copy(
                        hTc[:, o, rj * 128:(rj + 1) * 128], htp[:, 0:64].bitcast(bf16)
                    )
            # latent^T for these 256 rows
            for m in range(2):
                lps = eps_h.tile([128, 256], f32, name="lps", tag="hps")
                nk = HID // 128
                for k in range(nk):
                    nc.tensor.matmul(
                        lps[:, 0:256],
                        w2s[:, k, m * 128:(m + 1) * 128],
                        hTc[:, k, :],
                        start=(k == 0),
                        stop=(k == nk - 1 and not use_b2),
                    )
                if use_b2:
                    nc.tensor.matmul(
                        lps[:, 0:256],
                        b2s[:, m * 128:(m + 1) * 128],
                        ones1[:, 0:256],
                        start=False, stop=True,
                    )
                nc.vector.tensor_copy(
                    latT[:, m, rc * 256:(rc + 1) * 256], lps[:, 0:256]
                )

        for m in range(2):
            nc.vector.tensor_copy(residT[:, m, :], latT[:, m, :])
            nc.vector.tensor_copy(residTb[:, m, :], latT[:, m, :])

        enc_ctx.close()

        # =============== RVQ ===============
        vq_ctx = contextlib.ExitStack()
        vq_p = vq_ctx.enter_context(tc.tile_pool(name="vqp", bufs=1))
        vps_p = vq_ctx.enter_context(tc.tile_pool(name="vps", bufs=2, space="PSUM"))
        e2ts = vq_p.tile([128, 2, 2, VOCAB], bf16, name="e2ts")
        e2tp = vq_p.tile([128, VOCAB], mybir.dt.uint32, name="e2tp")
        qp = vq_p.tile([128, NSH], mybir.dt.uint32, name="qp")
        se2b = vq_p.tile([128, 2, VOCAB], bf16, name="se2b")
        from concourse import library_config
        nc.gpsimd.load_library(library_config.ap_gather)
        pk64 = persist_p.tile([128, RT * 4], f32, name="pk64")
        giota = const_p.tile([128, RT * 4], f32, name="giota")
        nc.sync.dma_start(giota[:], d_giota.ap())
        for lv in range(HQ):
            db = lv % 2
            for m in range(2):
                nc.sync.dma_start(
                    e2ts[:, db, m, :], e2t_d.ap()[lv, m * 128:(m + 1) * 128, :]
                )
            nc.sync.dma_start(se2b[:, db, :], se2_d.ap()[lv])
            nc.sync.dma_start(e2tp[:], e2tp_d.ap()[lv])
            qb = qp[:].bitcast(bf16).rearrange("p (n two) -> p n two", two=2)
            for qq in range(4):  # quarters of 4 row tiles, pipelined
                for rj in range(4):
                    rt = qq * 4 + rj
                    for g in range(4):
                        sps = vps_p.tile([128, 2048], f32, name="sps", tag="sps")
                        for cc in range(4):
                            c0 = g * 2048 + cc * 512
                            for k in range(2):
                                nc.tensor.matmul(
                                    sps[:, cc * 512:(cc + 1) * 512],
                                    residTb[:, k, rt * 128:(rt + 1) * 128],
                                    e2ts[:, db, k, c0:c0 + 512],
                                    start=(k == 0), stop=(k == 1),
                                )
                        nc.vector._custom_dve(
                            qpack,
                            out=trash[:],
                            in0=sps[:],
                            in1=se2b[:, db, g * 2048:(g + 1) * 2048],
                            imm2=BIG,
                            accum_out=pk64[:, rt * 4 + g: rt * 4 + g + 1],
                        )
                # index extraction for this quarter
                cs = qq * 16
                pk3 = pk64[:, cs:cs + 16].rearrange("p (a b) -> p a b", a=4)
                m16 = small_p.tile([128, 4], f32, name="m16", tag="m16")
                nc.vector.tensor_reduce(
                    m16[:], pk3, axis=mybir.AxisListType.X, op=AluOpType.max
                )
                msk = small_p.tile([128, 4, 4], f32, name="msk", tag="msk")
                nc.vector.tensor_tensor(
                    msk[:], pk3,
                    m16[:].rearrange("p (a o) -> p a o", o=1)
                    .broadcast_to((128, 4, 4)),
                    op=AluOpType.is_ge,
                )
                nc.vector.tensor_mul(
                    msk[:], msk[:],
                    giota[:, cs:cs + 16].rearrange("p (a b) -> p a b", a=4),
                )
                gidx = small_p.tile([128, 4], f32, name="gidx", tag="gidx")
                nc.vector.tensor_reduce(
                    gidx[:], msk[:], axis=mybir.AxisListType.X, op=AluOpType.add
                )
                nc.vector.tensor_scalar_min(gidx[:], gidx[:], 3.0)
                mi = small_p.tile([128, 4], mybir.dt.int32, name="mi", tag="mi")
                nc.vector.tensor_copy(mi[:], m16[:])
                nc.vector.tensor_scalar(
                    mi[:], mi[:], int(GRID) - 1, None, op0=AluOpType.bitwise_and
                )
                loc = small_p.tile([128, 4], f32, name="loc", tag="loc")
                nc.vector.tensor_copy(loc[:], mi[:])
                nc.vector.tensor_scalar(
                    gidx[:], gidx[:], GRID, None, op0=AluOpType.mult
                )
                nc.vector.tensor_add(loc[:], loc[:], gidx[:])
                nc.vector.tensor_copy(idx16[:, qq * 4:(qq + 1) * 4], loc[:])
                # stage indices (wrapped + replicated across 8 Q7 groups)
                for kk in range(8):
                    nc.gpsimd.dma_start(
                        idxg[0:16, qq * 4:(qq + 1) * 4, kk],
                        idx16[kk * 16:(kk + 1) * 16, qq * 4:(qq + 1) * 4],
                    )
                for gg in range(1, 8):
                    nc.gpsimd.dma_start(
                        idxg[gg * 16:(gg + 1) * 16, qq * 4:(qq + 1) * 4, :],
                        idxg[0:16, qq * 4:(qq + 1) * 4, :],
                    )
                nc.gpsimd.ap_gather(
                    qp[:, qq * 512:(qq + 1) * 512],
                    e2tp[:],
                    idxg[:, qq * 4:(qq + 1) * 4, :].rearrange("p a b -> p (a b)"),
                    channels=128, num_elems=VOCAB, d=1, num_idxs=512,
                )
                # apply the PREVIOUS quarter's update here so the in-order
                # DVE never waits on this quarter's gather
                for uq in ([qq - 1] if qq > 0 else []) + ([qq] if qq == 3 else []):
                    for m in range(2):
                        nc.vector.tensor_sub(
                            residT[:, m, uq * 512:(uq + 1) * 512]
                            .rearrange("p (n o) -> p n o", o=1),
                            residT[:, m, uq * 512:(uq + 1) * 512]
                            .rearrange("p (n o) -> p n o", o=1),
                            qb[:, uq * 512:(uq + 1) * 512, m:m + 1],
                        )
                        if lv < HQ - 1:
                            nc.vector.tensor_copy(
                                residTb[:, m, uq * 512:(uq + 1) * 512],
                                residT[:, m, uq * 512:(uq + 1) * 512],
                            )
            for m in range(2):
                nc.scalar.activation(
                    trash[:], residT[:, m, :], Square,
                    accum_out=out_sb[:, 2 * lv + m: 2 * lv + m + 1],
                )

        for m in range(2):
            # quant^T = latT - residT (stored back into latT)
            nc.vector.tensor_sub(latT[:, m, :], latT[:, m, :], residT[:, m, :])

        vq_ctx.close()

        # =============== decoder ===============
        dec_ctx = contextlib.ExitStack()
        dec_p = dec_ctx.enter_context(tc.tile_pool(name="decp", bufs=1))
        work_p = dec_ctx.enter_context(tc.tile_pool(name="decw", bufs=2))
        dps_p = dec_ctx.enter_context(tc.tile_pool(name="dps", bufs=4, space="PSUM"))
        dw1s = dec_p.tile([128, 2, HID], bf16, name="dw1s")
        quantTb = dec_p.tile([128, 2, NSH], bf16, name="quantTb")
        for m in range(2):
            nc.vector.tensor_copy(quantTb[:, m, :], latT[:, m, :])
        for k in range(2):
            nc.sync.dma_start(dw1s[:, k, :], dw1_d.ap()[k * 128:(k + 1) * 128, :])
        dw2s = dec_p.tile([128, HID // 128, OBS], bf16, name="dw2s")
        for k in range(HID // 128):
            nc.sync.dma_start(dw2s[:, k, :], dw2_d.ap()[k * 128:(k + 1) * 128, :])
        db1s = const_p.tile([HID // 512, 512], bf16, name="db1s")
        if use_db1:
            nc.sync.dma_start(db1s[:], db1_d.ap())
        db2s = const_p.tile([OBS // 512, 512], bf16, name="db2s")
        if use_db2:
            nc.sync.dma_start(db2s[:], db2_d.ap())

        for rc in range(4):  # row chunks of 512
            dhT = work_p.tile([128, HID // 128, 512], bf16, name="dhT", tag="hT")
            for ht in range(HID // 128):
                dps = dps_p.tile([128, 512], f32, name="dps", tag="dmm")
                for k in range(2):
                    nc.tensor.matmul(
                        dps[:, 0:512],
                        dw1s[:, k, ht * 128:(ht + 1) * 128],
                        quantTb[:, k, rc * 512:(rc + 1) * 512],
                        start=(k == 0), stop=(k == 1 and not use_db1),
                    )
                if use_db1:
                    nc.tensor.matmul(
                        dps[:, 0:512],
                        db1s[(ht * 128) // 512:(ht * 128) // 512 + 1,
                             (ht * 128) % 512:(ht * 128) % 512 + 128],
                        ones1[:],
                        start=False, stop=True,
                    )
                nc.scalar.activation(dhT[:, ht, :], dps[:, 0:512], Relu)
            for ot in range(OBS // 128):
                xTl = work_p.tile([128, 512], bf16, name="xTl", tag="xTl")
                nc.sync.dma_start(
                    xTl[:],
                    x_d.ap()[ot * 128:(ot + 1) * 128, rc * 512:(rc + 1) * 512],
                )
                rps = dps_p.tile([128, 512], f32, name="rps", tag="dmm")
                nk = HID // 128
                for k in range(nk):
                    nc.tensor.matmul(
                        rps[:, 0:512],
                        dw2s[:, k, ot * 128:(ot + 1) * 128],
                        dhT[:, k, :],
                        start=(k == 0), stop=(k == nk - 1 and not use_db2),
                    )
                if use_db2:
                    nc.tensor.matmul(
                        rps[:, 0:512],
                        db2s[(ot * 128) // 512:(ot * 128) // 512 + 1,
                             (ot * 128) % 512:(ot * 128) % 512 + 128],
                        ones1[:],
                        start=False, stop=True,
                    )
                diff = work_p.tile([128, 512], f32, name="diff", tag="diff")
                nc.vector.tensor_sub(diff[:], rps[:, 0:512], xTl[:])
                nc.scalar.activation(
                    diff[:], diff[:], Square,
                    accum_out=out_sb[:, 8 + rc * 8 + ot: 9 + rc * 8 + ot],
                )

        dec_ctx.close()
        nc.sync.dma_start(out_d.ap(), out_sb[:])

    nc.compile()
    return nc


def _host_prep(inputs):
    import ml_dtypes

    x = np.asarray(inputs["x"], np.float32)
    cb = np.ascontiguousarray(np.asarray(inputs["codebooks"], np.float32))
    w1 = np.ascontiguousarray(np.asarray(inputs["enc_w1"], np.float32))
    b1 = np.asarray(inputs["enc_b1"], np.float32)
    lng = np.asarray(inputs["ln_g"], np.float32)
    lnb = np.asarray(inputs["ln_b"], np.float32)
    w2 = np.asarray(inputs["enc_w2"], np.float32)
    b2 = np.asarray(inputs["enc_b2"], np.float32)
    dw1 = np.ascontiguousarray(np.asarray(inputs["dec_w1"], np.float32))
    db1 = np.asarray(inputs["dec_b1"], np.float32)
    dw2 = np.asarray(inputs["dec_w2"], np.float32)
    db2 = np.asarray(inputs["dec_b2"], np.float32)

    assert np.all(lnb == 0.0) and np.all(lng > 0.0), "kernel assumes ln_b==0, ln_g>0"
    w2g = w2 * lng[:, None]  # relu(z*g)@W2 == relu(z)@(g[:,None]*W2) for g>0

    # sample-estimate per-level score ranges to pick K, SHIFT
    rng = np.random.default_rng(0)
    sel = rng.choice(x.shape[0], 256, replace=False)
    h = x[sel] @ w1 + b1
    mu = h.mean(-1, keepdims=True)
    var = ((h - mu) ** 2).mean(-1, keepdims=True)
    h = np.maximum((h - mu) / np.sqrt(var + LN_EPS) * lng + lnb, 0.0)
    resid = h @ w2 + b2
    e2sum = (cb.astype(np.float64) ** 2).sum(-1).astype(np.float32)  # [HQ, VOCAB]
    Ks, SHIFTs = [], []
    for lv in range(HQ):
        sc = 2.0 * resid @ cb[lv].T - e2sum[lv]
        lo, hi = float(sc.min()), float(sc.max())
        span = hi - lo
        shift = -lo + 0.75 * span + 16.0       # margin: scores stay well positive
        smax = (hi + shift) * 2.0              # 2x safety for sample underestimate
        K = np.float32((2.0**24 * 0.98) / smax)
        Ks.append(K)
        SHIFTs.append(np.float32(shift))
        idx = sc.argmax(-1)
        resid = resid - cb[lv][idx]

    e2t = cb.transpose(0, 2, 1)  # [HQ, LAT, VOCAB]
    e2t_bf = np.zeros((HQ, LAT, VOCAB), ml_dtypes.bfloat16)
    se2 = np.zeros((HQ, 128, VOCAB), ml_dtypes.bfloat16)
    e2tp_pack = np.zeros((HQ, 128, VOCAB), np.uint32)
    for lv in range(HQ):
        e2t_bf[lv] = (np.float32(2.0 * Ks[lv]) * e2t[lv]).astype(ml_dtypes.bfloat16)
        row = (Ks[lv] * (SHIFTs[lv] - e2sum[lv])).astype(ml_dtypes.bfloat16)
        se2[lv] = np.broadcast_to(row, (128, VOCAB))
        pk0 = e2t[lv, :128].astype(ml_dtypes.bfloat16).view(np.uint16).astype(np.uint32)
        pk1 = e2t[lv, 128:].astype(ml_dtypes.bfloat16).view(np.uint16).astype(np.uint32)
        e2tp_pack[lv] = pk0 | (pk1 << 16)

    common = {
        "w1b": np.ascontiguousarray(w1.astype(ml_dtypes.bfloat16)),
        "b1": np.ascontiguousarray(b1.reshape(HID // 512, 512).astype(ml_dtypes.bfloat16)),
        "w2b": np.ascontiguousarray(w2g.astype(ml_dtypes.bfloat16)),
        "b2": b2.reshape(1, LAT).astype(ml_dtypes.bfloat16),
        "dw1b": np.ascontiguousarray(dw1.astype(ml_dtypes.bfloat16)),
        "db1": np.ascontiguousarray(db1.reshape(HID // 512, 512).astype(ml_dtypes.bfloat16)),
        "dw2b": np.ascontiguousarray(dw2.astype(ml_dtypes.bfloat16)),
        "db2": np.ascontiguousarray(db2.reshape(OBS // 512, 512).astype(ml_dtypes.bfloat16)),
        "e2t": np.ascontiguousarray(e2t_bf),
        "se2": np.ascontiguousarray(se2),
        "e2tp": e2tp_pack,
        "identb": np.eye(128, dtype=np.float32).astype(ml_dtypes.bfloat16),
        "giota": np.ascontiguousarray(
            np.tile(np.arange(4, dtype=np.float32), (128, RT))
        ),
    }
    flags = dict(
        use_b1=bool(np.any(b1 != 0)),
        use_b2=bool(np.any(b2 != 0)),
        use_db1=bool(np.any(db1 != 0)),
        use_db2=bool(np.any(db2 != 0)),
    )
    in_maps = []
    for c in range(NCORES):
        m = dict(common)
        m["xbt"] = np.ascontiguousarray(
            x[c * NSH:(c + 1) * NSH].T.astype(ml_dtypes.bfloat16)
        )
        in_maps.append(m)
    return in_maps, flags


def _combine(results):
    rlv = rrec = 0.0
    for c in range(NCORES):
        o = np.asarray(results[c]["out"], np.float64)
        rlv += o[:, 0:8].sum()
        rrec += o[:, 8:40].sum()
    return np.float32(1.5 * rlv / (N * LAT) + 0.5 * rrec / (N * OBS))


_NC_CACHE = {}


def get_nc(flags):
    key = tuple(sorted(flags.items()))
    if key not in _NC_CACHE:
        _NC_CACHE[key] = build_nc(**flags)
    return _NC_CACHE[key]


def kernel(**inputs) -> np.ndarray:
    in_maps, flags = _host_prep(inputs)
    nc = get_nc(flags)
    res = run_bass_kernel_spmd(nc, in_maps, core_ids=list(range(NCORES)))
    return _combine(res.results)



# revision 5
# speedup vs baseline: 1.1642x; 1.1642x over previous
# Trainium2 Bass kernel for residual-VQ autoencoder loss (vq_codebook).
# Data-parallel over rows: 8 NeuronCores, 2048 rows each; codebooks/weights
# replicated. The scalar loss is assembled on host from per-core [128,40]
# partial-sum outputs (no on-chip collectives needed).
#
# Per core (RT=16 row-tiles of 128):
#   encoder: h = x@W1+b1 -> LN -> ReLU -> latent = h@W2' + b2 (ln_g folded
#   into W2; a host-side rotation R is folded into W2/dec_w1/codebooks so
#   latent dim 255 carries minimal residual variance).
#   RVQ pair-max argmax:
#     vocab columns are split even/odd; per scan-group of 1024 pairs the PE
#     writes evens to PSUM-E and odds to PSUM-O (bf16 matmuls). The score
#     bias K*(SHIFT-|E|^2) rides contraction row 255 (the sacrificed dim:
#     residTb row 127 of the k=1 tile is constant 1.0).
#     ACT copies PSUM-O -> SBUF f32 (PSUM has one DVE read port); the custom
#     DVE op QPACK2 then computes per element
#       packed = quant2048(max(odd,even)) + 2*i + (odd>=even)
#     and MAX-accumulates => packed argmax over the group; local idx = packed
#     mod 2048 directly equals the within-group vocab index.
#     idx -> q = codebook[idx] via gpsimd.dma_gather (exact rotated bf16);
#     resid -= q^T.
#   loss telescopes: sum_l 1.5*mean((q_l-r_l)^2) = 1.5*(R0-R4)/(N*LAT);
#   quant^T = latent^T - resid^T; decoder stays feature-major;
#   Rrec = sum((recon-x)^2) using x^T spilled to DRAM during the encoder.

import sys

sys.path.insert(0, "/opt/trn_rl_repo")

import numpy as np

import concourse.bass as bass
import concourse.mybir as mybir
import concourse.bacc as bacc
import concourse.tile as tile
from concourse.alu_op_type import AluOpType
from concourse.bass_utils import run_bass_kernel_spmd

OBS, HID, LAT = 1024, 2048, 256
VOCAB, HQ = 8192, 4
N, NCORES = 16384, 8
NSH = N // NCORES          # 2048 rows per core
RT = NSH // 128            # 16 row tiles
LN_EPS = 1e-5
GRID = 2048.0               # 11 bits: 2*i + b per 1024-pair scan group
BIG = float(1.5 * 2.0**34)  # ulp(BIG)=2048 -> (x+BIG)-BIG rounds to mult of GRID

f32 = mybir.dt.float32
f32r = mybir.dt.float32r
bf16 = mybir.dt.bfloat16
i16 = mybir.dt.int16

_QPACK2 = None


def _register_qpack2():
    """packed = quant2048(max(Src0,Src1)) + 2*idx + (Src0>=Src1);
    accum_out = max over free dim.  Src0: odd scores (SBUF f32, ACT-copied
    from PSUM); Src1: even scores (PSUM f32).  Scores are pre-biased in the
    matmul (bias rides contraction row 255).  With Src0=odds, the local
    vocab index within the 2048-wide group is exactly packed mod 2048."""
    global _QPACK2
    if _QPACK2 is not None:
        return _QPACK2
    from concourse import dve_ops
    from concourse.dve_spec import (
        Spec, Src0, Src1, C1, C2, AluOp, Idx, lower, Bin, Zero, Scan,
    )
    from concourse.dve_table_gen import DveOpSpec

    def _ref(in0, in1, s0, s1, imm2):
        o = np.asarray(in0, np.float32)
        e = np.asarray(in1, np.float32)
        m = np.maximum(o, e)
        q = (m + np.float32(imm2)).astype(np.float32) - np.float32(imm2)
        idx2 = np.arange(o.shape[-1], dtype=np.float32) * np.float32(s1)
        out = ((q + idx2) + (o >= e).astype(np.float32)).astype(np.float32)
        acc = out.max(axis=-1, keepdims=True)
        return out, acc

    m = Bin(AluOp.MAX, Src0, Src1)
    q = Bin(AluOp.SUBTRACT, Bin(AluOp.ADD, m, C2), C2)
    idx2 = Scan(AluOp.ADD, C1, init=Bin(AluOp.SUBTRACT, Zero, C1))
    g = Bin(AluOp.IS_GE, Src0, Src1)
    body = Bin(AluOp.ADD, Bin(AluOp.ADD, q, idx2), g)
    spec = Spec(body=body, accum=AluOp.MAX, reference=_ref)
    op = dve_ops.DveOp("QPACK2_PAIRMAX", spec, subdim=False, uops_sha={})
    dve_ops.OPS.append(op)
    dve_ops.CUSTOM_DVE_SPECS[op.name] = op.spec
    dve_ops._SUB_OPCODE_FOR_NAME[op.name] = (
        dve_ops._CUSTOM_DVE_ROW_BASE + len(dve_ops.OPS) - 1
    )
    for ver in ("v3", "v4"):
        s = DveOpSpec(
            name=op.name,
            opcode=dve_ops.get_dve_sub_opcode(op.name),
            uops=lower(spec, ver=ver),
            rd1_en=True,
        )
        op.uops_sha[ver] = s.sha(ver)
    _QPACK2 = op
    return op


def build_nc(use_b1=True, use_b2=True, use_db1=True, use_db2=True):
    qpack2 = _register_qpack2()
    nc = bacc.Bacc(
        "TRN2",
        target_bir_lowering=False,
        debug=False,
        enable_asserts=False,
        num_devices=NCORES,
    )
    Relu = mybir.ActivationFunctionType.Relu
    Square = mybir.ActivationFunctionType.Square
    Sqrt = mybir.ActivationFunctionType.Sqrt
    Copy = mybir.ActivationFunctionType.Copy

    # ---------------- DRAM I/O ----------------
    x_d = nc.dram_tensor("xbt", [OBS, NSH], bf16, kind="ExternalInput")
    w1_d = nc.dram_tensor("w1b", [OBS, HID], bf16, kind="ExternalInput")
    b1_d = nc.dram_tensor("b1", [HID // 512, 512], bf16, kind="ExternalInput")
    w2_d = nc.dram_tensor("w2b", [HID, LAT], bf16, kind="ExternalInput")
    b2_d = nc.dram_tensor("b2", [1, LAT], bf16, kind="ExternalInput")
    dw1_d = nc.dram_tensor("dw1b", [LAT, HID], bf16, kind="ExternalInput")
    db1_d = nc.dram_tensor("db1", [HID // 512, 512], bf16, kind="ExternalInput")
    dw2_d = nc.dram_tensor("dw2b", [HID, OBS], bf16, kind="ExternalInput")
    db2_d = nc.dram_tensor("db2", [OBS // 512, 512], bf16, kind="ExternalInput")
    # even/odd codebooks: [HQ, k, 128, VOCAB//2]; row (k=1,p=127) = bias
    e2e_d = nc.dram_tensor("e2e", [HQ, 2, 128, VOCAB // 2], bf16, kind="ExternalInput")
    e2o_d = nc.dram_tensor("e2o", [HQ, 2, 128, VOCAB // 2], bf16, kind="ExternalInput")
    e2tp_d = nc.dram_tensor("e2tp", [HQ, 128, VOCAB], mybir.dt.uint32, kind="ExternalInput")
    identb_d = nc.dram_tensor("identb", [128, 128], bf16, kind="ExternalInput")
    ones_d = nc.dram_tensor("onesrow", [1, NSH], bf16, kind="ExternalInput")
    d_giota = nc.dram_tensor("giota", [128, RT * 4], f32, kind="ExternalInput")
    out_d = nc.dram_tensor("out", [128, 40], f32, kind="ExternalOutput")

    import contextlib

    with tile.TileContext(nc) as tc, contextlib.ExitStack() as ctx:
        const_p = ctx.enter_context(tc.tile_pool(name="const", bufs=1))
        persist_p = ctx.enter_context(tc.tile_pool(name="persist", bufs=1))
        small_p = ctx.enter_context(tc.tile_pool(name="small", bufs=4))

        # ---- constants ----
        identb = const_p.tile([128, 128], bf16, name="identb")
        nc.sync.dma_start(identb[:], identb_d.ap())
        ones1 = const_p.tile([1, 512], bf16, name="ones1")
        nc.vector.memset(ones1[:], 1.0)
        out_sb = const_p.tile([128, 40], f32, name="out_sb")
        nc.vector.memset(out_sb[:], 0.0)
        epsc = const_p.tile([128, 1], f32, name="epsc")
        nc.vector.memset(epsc[:], LN_EPS)

        # ---- persistent ----
        latT = persist_p.tile([128, 2, NSH], f32, name="latT")
        residT = persist_p.tile([128, 2, NSH], f32, name="residT")
        residTb = persist_p.tile([128, 2, NSH], bf16, name="residTb")
        trash = persist_p.tile([128, 2048], bf16, name="trash")
        idx16 = persist_p.tile([128, RT], i16, name="idx16")
        idxg = persist_p.tile([128, RT, 8], i16, name="idxg")
        nc.vector.memset(idxg[:], 0)
        # contraction row 255 of the scores is the bias row: resid slot = 1.0
        # (engine ops can't start at partition 127; write it via DMA)
        nc.sync.dma_start(residTb[127:128, 1, :], ones_d.ap())
        # =============== encoder ===============
        enc_ctx = contextlib.ExitStack()
        enc_p = enc_ctx.enter_context(tc.tile_pool(name="encp", bufs=1))
        work_p = enc_ctx.enter_context(tc.tile_pool(name="encw", bufs=2))
        eps_h = enc_ctx.enter_context(tc.tile_pool(name="epsh", bufs=6, space="PSUM"))
        eps_t = enc_ctx.enter_context(tc.tile_pool(name="epst", bufs=2, space="PSUM"))
        w2s = enc_p.tile([128, HID // 128, LAT], bf16, name="w2s")
        w1s = enc_p.tile([128, OBS // 128, HID], bf16, name="w1s")
        for k in range(OBS // 128):
            nc.sync.dma_start(w1s[:, k, :], w1_d.ap()[k * 128:(k + 1) * 128, :])
        for k in range(HID // 128):
            nc.sync.dma_start(w2s[:, k, :], w2_d.ap()[k * 128:(k + 1) * 128, :])
        b1s = const_p.tile([HID // 512, 512], bf16, name="b1s")
        if use_b1:
            nc.sync.dma_start(b1s[:], b1_d.ap())
        b2s = const_p.tile([1, LAT], bf16, name="b2s")
        if use_b2:
            nc.sync.dma_start(b2s[:], b2_d.ap())

        xT_sb = enc_p.tile([128, OBS // 128, NSH], bf16, name="xT_sb")
        for k in range(OBS // 128):
            nc.sync.dma_start(xT_sb[:, k, :], x_d.ap()[k * 128:(k + 1) * 128, :])
        for rc in range(8):  # row chunks of 256 (2 row tiles each)
            hTc = work_p.tile([128, HID // 128, 256], bf16, name="hTc", tag="hT")
            for rj in range(2):
                rt = rc * 2 + rj
                # h = x @ W1 (+ b1), chunk-grained PSUM for overlap
                hsb = work_p.tile([128, 2048], f32, name="hsb", tag="hsb")
                bns = small_p.tile([128, 4, 6], f32, name="bns", tag="s1")
                for cc in range(4):
                    hps = eps_h.tile([128, 512], f32, name="hps", tag="hps")
                    nmm = OBS // 128
                    for k in range(nmm):
                        nc.tensor.matmul(
                            hps[:],
                            xT_sb[:, k, rt * 128:(rt + 1) * 128],
                            w1s[:, k, cc * 512:(cc + 1) * 512],
                            start=(k == 0),
                            stop=(k == nmm - 1 and not use_b1),
                        )
                    if use_b1:
                        nc.tensor.matmul(
                            hps[:],
                            ones1[:, 0:128],
                            b1s[cc:cc + 1, :],
                            start=False, stop=True,
                        )
                    nc.scalar.activation(
                        hsb[:, cc * 512:(cc + 1) * 512], hps[:],
                        mybir.ActivationFunctionType.Copy,
                    )
                    nc.vector.bn_stats(bns[:, cc, :], hps[:])
                mv = small_p.tile([128, 2], f32, name="mv", tag="s2")
                nc.vector.bn_aggr(mv[:], bns[:].rearrange("p a b -> p (a b)"))
                std = small_p.tile([128, 1], f32, name="std", tag="s6")
                nc.scalar.activation(std[:], mv[:, 1:2], Sqrt, bias=epsc[:])
                rstd = small_p.tile([128, 1], f32, name="rstd", tag="s7")
                nc.vector.reciprocal(rstd[:], std[:])
                nmr = small_p.tile([128, 1], f32, name="nmr", tag="s8")
                nc.vector.tensor_scalar(
                    nmr[:], mv[:, 0:1], rstd[:], -1.0,
                    op0=AluOpType.mult, op1=AluOpType.mult,
                )
                hrelu = work_p.tile([128, 2048], bf16, name="hrelu", tag="hrelu")
                nc.scalar.activation(
                    hrelu[:], hsb[:], Relu, bias=nmr[:], scale=rstd[:]
                )
                for o in range(HID // 128):
                    htp = eps_t.tile([128, 128], f32, name="htp", tag="tp")
                    nc.tensor.matmul(
                        htp[:, 0:64].bitcast(bf16),
                        hrelu[:, o * 128:(o + 1) * 128],
                        identb[:], is_transpose=True, start=True, stop=True,
                    )
                    nc.vector.tensor_copy(
                        hTc[:, o, rj * 128:(rj + 1) * 128], htp[:, 0:64].bitcast(bf16)
                    )
            # latent^T for these 256 rows
            for m in range(2):
                lps = eps_h.tile([128, 256], f32, name="lps", tag="hps")
                nk = HID // 128
                for k in range(nk):
                    nc.tensor.matmul(
                        lps[:, 0:256],
                        w2s[:, k, m * 128:(m + 1) * 128],
                        hTc[:, k, :],
                        start=(k == 0),
                        stop=(k == nk - 1 and not use_b2),
                    )
                if use_b2:
                    nc.tensor.matmul(
                        lps[:, 0:256],
                        b2s[:, m * 128:(m + 1) * 128],
                        ones1[:, 0:256],
                        start=False, stop=True,
                    )
                nc.vector.tensor_copy(
                    latT[:, m, rc * 256:(rc + 1) * 256], lps[:, 0:256]
                )

        nc.vector.tensor_copy(residT[:, 0, :], latT[:, 0, :])
        nc.vector.tensor_copy(residT[:, 1, :], latT[:, 1, :])
        nc.vector.tensor_copy(residTb[:, 0, :], latT[:, 0, :])
        nc.vector.tensor_copy(residTb[0:127, 1, :], latT[0:127, 1, :])

        enc_ctx.close()

        # =============== RVQ ===============
        vq_ctx = contextlib.ExitStack()
        vq_p = vq_ctx.enter_context(tc.tile_pool(name="vqp", bufs=1))
        vps_p = vq_ctx.enter_context(tc.tile_pool(name="vps", bufs=2, space="PSUM"))
        sod_p = vq_ctx.enter_context(tc.tile_pool(name="sod", bufs=3))
        # [p, buf, k, c]: double-buffered even/odd codebooks
        e2es = vq_p.tile([128, 2, 2, VOCAB // 2], bf16, name="e2es")
        e2os = vq_p.tile([128, 2, 2, VOCAB // 2], bf16, name="e2os")
        e2tp = vq_p.tile([128, 2, VOCAB], mybir.dt.uint32, name="e2tp")
        qp = vq_p.tile([128, NSH], mybir.dt.uint32, name="qp")
        from concourse import library_config
        nc.gpsimd.load_library(library_config.ap_gather)
        pk64 = persist_p.tile([128, RT * 4], f32, name="pk64")
        giota = const_p.tile([128, RT * 4], f32, name="giota")
        nc.sync.dma_start(giota[:], d_giota.ap())

        def load_level(lv):
            db = lv % 2
            for k in range(2):
                nc.sync.dma_start(e2es[:, db, k, :], e2e_d.ap()[lv, k])
                nc.sync.dma_start(e2os[:, db, k, :], e2o_d.ap()[lv, k])
            nc.sync.dma_start(e2tp[:, db, :], e2tp_d.ap()[lv])

        load_level(0)
        for lv in range(HQ):
            db = lv % 2
            if lv + 1 < HQ:
                load_level(lv + 1)
            qb = qp[:].bitcast(bf16).rearrange("p (n two) -> p n two", two=2)
            for qq in range(4):  # quarters of 4 row tiles, pipelined
                for rj in range(4):
                    rt = qq * 4 + rj
                    for g in range(4):
                        psE = vps_p.tile([128, 1024], f32, name="psE", tag="psE")
                        psO = vps_p.tile([128, 1024], f32, name="psO", tag="psO")
                        sodd = sod_p.tile([128, 1024], f32, name="sodd", tag="sodd")
                        for h in range(2):
                            c0 = g * 1024 + h * 512
                            for k in range(2):
                                nc.tensor.matmul(
                                    psO[:, h * 512:(h + 1) * 512],
                                    residTb[:, k, rt * 128:(rt + 1) * 128],
                                    e2os[:, db, k, c0:c0 + 512],
                                    start=(k == 0), stop=(k == 1),
                                )
                        for h in range(2):
                            c0 = g * 1024 + h * 512
                            for k in range(2):
                                nc.tensor.matmul(
                                    psE[:, h * 512:(h + 1) * 512],
                                    residTb[:, k, rt * 128:(rt + 1) * 128],
                                    e2es[:, db, k, c0:c0 + 512],
                                    start=(k == 0), stop=(k == 1),
                                )
                        nc.scalar.activation(sodd[:], psO[:], Copy)
                        nc.vector._custom_dve(
                            qpack2,
                            out=trash[:, 0:1024],
                            in0=sodd[:],
                            in1=psE[:],
                            s1=2.0,
                            imm2=BIG,
                            accum_out=pk64[:, rt * 4 + g: rt * 4 + g + 1],
                        )
                # index extraction for this quarter
                cs = qq * 16
                pk3 = pk64[:, cs:cs + 16].rearrange("p (a b) -> p a b", a=4)
                m16 = small_p.tile([128, 4], f32, name="m16", tag="m16")
                nc.vector.tensor_reduce(
                    m16[:], pk3, axis=mybir.AxisListType.X, op=AluOpType.max
                )
                msk = small_p.tile([128, 4, 4], f32, name="msk", tag="msk")
                nc.vector.tensor_tensor(
                    msk[:], pk3,
                    m16[:].rearrange("p (a o) -> p a o", o=1)
                    .broadcast_to((128, 4, 4)),
                    op=AluOpType.is_ge,
                )
                nc.vector.tensor_mul(
                    msk[:], msk[:],
                    giota[:, cs:cs + 16].rearrange("p (a b) -> p a b", a=4),
                )
                gidx = small_p.tile([128, 4], f32, name="gidx", tag="gidx")
                nc.vector.tensor_reduce(
                    gidx[:], msk[:], axis=mybir.AxisListType.X, op=AluOpType.add
                )
                nc.vector.tensor_scalar_min(gidx[:], gidx[:], 3.0)
                mi = small_p.tile([128, 4], mybir.dt.int32, name="mi", tag="mi")
                nc.vector.tensor_copy(mi[:], m16[:])
                nc.vector.tensor_scalar(
                    mi[:], mi[:], int(GRID) - 1, None, op0=AluOpType.bitwise_and
                )
                loc = small_p.tile([128, 4], f32, name="loc", tag="loc")
                nc.vector.tensor_copy(loc[:], mi[:])
                nc.vector.tensor_scalar(
                    gidx[:], gidx[:], GRID, None, op0=AluOpType.mult
                )
                nc.vector.tensor_add(loc[:], loc[:], gidx[:])
                nc.vector.tensor_copy(idx16[:, qq * 4:(qq + 1) * 4], loc[:])
                # stage indices (wrapped + replicated across 8 Q7 groups)
                for kk in range(8):
                    nc.gpsimd.dma_start(
                        idxg[0:16, qq * 4:(qq + 1) * 4, kk],
                        idx16[kk * 16:(kk + 1) * 16, qq * 4:(qq + 1) * 4],
                    )
                for gg in range(1, 8):
                    nc.gpsimd.dma_start(
                        idxg[gg * 16:(gg + 1) * 16, qq * 4:(qq + 1) * 4, :],
                        idxg[0:16, qq * 4:(qq + 1) * 4, :],
                    )
                nc.gpsimd.ap_gather(
                    qp[:, qq * 512:(qq + 1) * 512],
                    e2tp[:, db, :],
                    idxg[:, qq * 4:(qq + 1) * 4, :].rearrange("p a b -> p (a b)"),
                    channels=128, num_elems=VOCAB, d=1, num_idxs=512,
                )
                # apply the PREVIOUS quarter's update here so the in-order
                # DVE never waits on this quarter's gather
                for uq in ([qq - 1] if qq > 0 else []) + ([qq] if qq == 3 else []):
                    for m in range(2):
                        nc.vector.tensor_sub(
                            residT[:, m, uq * 512:(uq + 1) * 512]
                            .rearrange("p (n o) -> p n o", o=1),
                            residT[:, m, uq * 512:(uq + 1) * 512]
                            .rearrange("p (n o) -> p n o", o=1),
                            qb[:, uq * 512:(uq + 1) * 512, m:m + 1],
                        )
                        if lv < HQ - 1:
                            if m == 0:
                                nc.vector.tensor_copy(
                                    residTb[:, 0, uq * 512:(uq + 1) * 512],
                                    residT[:, 0, uq * 512:(uq + 1) * 512],
                                )
                            else:
                                nc.vector.tensor_copy(
                                    residTb[0:127, 1, uq * 512:(uq + 1) * 512],
                                    residT[0:127, 1, uq * 512:(uq + 1) * 512],
                                )
            for m in range(2):
                nc.scalar.activation(
                    trash[:], residT[:, m, :], Square,
                    accum_out=out_sb[:, 2 * lv + m: 2 * lv + m + 1],
                )

        for m in range(2):
            # quant^T = latT - residT (stored back into latT)
            nc.vector.tensor_sub(latT[:, m, :], latT[:, m, :], residT[:, m, :])

        vq_ctx.close()

        # =============== decoder ===============
        dec_ctx = contextlib.ExitStack()
        dec_p = dec_ctx.enter_context(tc.tile_pool(name="decp", bufs=1))
        work_p = dec_ctx.enter_context(tc.tile_pool(name="decw", bufs=2))
        dps_p = dec_ctx.enter_context(tc.tile_pool(name="dps", bufs=4, space="PSUM"))
        dw1s = dec_p.tile([128, 2, HID], bf16, name="dw1s")
        quantTb = dec_p.tile([128, 2, NSH], bf16, name="quantTb")
        for m in range(2):
            nc.vector.tensor_copy(quantTb[:, m, :], latT[:, m, :])
        for k in range(2):
            nc.sync.dma_start(dw1s[:, k, :], dw1_d.ap()[k * 128:(k + 1) * 128, :])
        dw2s = dec_p.tile([128, HID // 128, OBS], bf16, name="dw2s")
        for k in range(HID // 128):
            nc.sync.dma_start(dw2s[:, k, :], dw2_d.ap()[k * 128:(k + 1) * 128, :])
        db1s = const_p.tile([HID // 512, 512], bf16, name="db1s")
        if use_db1:
            nc.sync.dma_start(db1s[:], db1_d.ap())
        db2s = const_p.tile([OBS // 512, 512], bf16, name="db2s")
        if use_db2:
            nc.sync.dma_start(db2s[:], db2_d.ap())

        for rc in range(4):  # row chunks of 512
            dhT = work_p.tile([128, HID // 128, 512], bf16, name="dhT", tag="hT")
            for ht in range(HID // 128):
                dps = dps_p.tile([128, 512], f32, name="dps", tag="dmm")
                for k in range(2):
                    nc.tensor.matmul(
                        dps[:, 0:512],
                        dw1s[:, k, ht * 128:(ht + 1) * 128],
                        quantTb[:, k, rc * 512:(rc + 1) * 512],
                        start=(k == 0), stop=(k == 1 and not use_db1),
                    )
                if use_db1:
                    nc.tensor.matmul(
                        dps[:, 0:512],
                        db1s[(ht * 128) // 512:(ht * 128) // 512 + 1,
                             (ht * 128) % 512:(ht * 128) % 512 + 128],
                        ones1[:],
                        start=False, stop=True,
                    )
                nc.scalar.activation(dhT[:, ht, :], dps[:, 0:512], Relu)
            for ot in range(OBS // 128):
                xTl = work_p.tile([128, 512], bf16, name="xTl", tag="xTl")
                nc.sync.dma_start(
                    xTl[:],
                    x_d.ap()[ot * 128:(ot + 1) * 128, rc * 512:(rc + 1) * 512],
                )
                rps = dps_p.tile([128, 512], f32, name="rps", tag="dmm")
                nk = HID // 128
                for k in range(nk):
                    nc.tensor.matmul(
                        rps[:, 0:512],
                        dw2s[:, k, ot * 128:(ot + 1) * 128],
                        dhT[:, k, :],
                        start=(k == 0), stop=(k == nk - 1 and not use_db2),
                    )
                if use_db2:
                    nc.tensor.matmul(
                        rps[:, 0:512],
                        db2s[(ot * 128) // 512:(ot * 128) // 512 + 1,
                             (ot * 128) % 512:(ot * 128) % 512 + 128],
                        ones1[:],
                        start=False, stop=True,
                    )
                diff = work_p.tile([128, 512], f32, name="diff", tag="diff")
                nc.vector.tensor_sub(diff[:], rps[:, 0:512], xTl[:])
                nc.scalar.activation(
                    diff[:], diff[:], Square,
                    accum_out=out_sb[:, 8 + rc * 8 + ot: 9 + rc * 8 + ot],
                )

        dec_ctx.close()
        nc.sync.dma_start(out_d.ap(), out_sb[:])

    nc.compile()
    return nc


def _host_prep(inputs):
    import ml_dtypes

    x = np.asarray(inputs["x"], np.float32)
    cb = np.ascontiguousarray(np.asarray(inputs["codebooks"], np.float32))
    w1 = np.ascontiguousarray(np.asarray(inputs["enc_w1"], np.float32))
    b1 = np.asarray(inputs["enc_b1"], np.float32)
    lng = np.asarray(inputs["ln_g"], np.float32)
    lnb = np.asarray(inputs["ln_b"], np.float32)
    w2 = np.asarray(inputs["enc_w2"], np.float32)
    b2 = np.asarray(inputs["enc_b2"], np.float32)
    dw1 = np.ascontiguousarray(np.asarray(inputs["dec_w1"], np.float32))
    db1 = np.asarray(inputs["dec_b1"], np.float32)
    dw2 = np.asarray(inputs["dec_w2"], np.float32)
    db2 = np.asarray(inputs["dec_b2"], np.float32)

    assert np.all(lnb == 0.0) and np.all(lng > 0.0), "kernel assumes ln_b==0, ln_g>0"
    w2g = w2 * lng[:, None]  # relu(z*g)@W2 == relu(z)@(g[:,None]*W2) for g>0

    e2sum = (cb.astype(np.float64) ** 2).sum(-1).astype(np.float32)  # [HQ, VOCAB]

    # sample rows: estimate per-level score ranges AND the residual covariance
    # (for the rotation that minimizes variance of the sacrificed dim 255)
    rng = np.random.default_rng(0)
    sel = rng.choice(x.shape[0], 1024, replace=False)
    h = x[sel] @ w1 + b1
    mu = h.mean(-1, keepdims=True)
    var = ((h - mu) ** 2).mean(-1, keepdims=True)
    h = np.maximum((h - mu) / np.sqrt(var + LN_EPS) * lng + lnb, 0.0)
    lat_s = h @ w2 + b2
    resid = lat_s.copy()
    pool = [resid.copy()]
    Ks, SHIFTs = [], []
    for lv in range(HQ):
        sc = 2.0 * resid @ cb[lv].T - e2sum[lv]
        lo, hi = float(sc.min()), float(sc.max())
        span = hi - lo
        shift = -lo + 0.75 * span + 16.0       # margin: scores stay well positive
        smax = (hi + shift) * 2.0              # 2x safety for sample underestimate
        K = np.float32((2.0**24 * 0.98) / smax)
        Ks.append(K)
        SHIFTs.append(np.float32(shift))
        idx = sc.argmax(-1)
        resid = resid - cb[lv][idx]
        if lv < HQ - 1:
            pool.append(resid.copy())
    P = np.concatenate(pool, 0)
    C = np.cov(P.T)
    w_eig, V = np.linalg.eigh(C)               # ascending eigenvalues
    rot = np.ascontiguousarray(V[:, ::-1])     # col 255 = min-variance direction

    cbr = np.ascontiguousarray(cb @ rot)       # rotated codebooks [HQ, V, LAT]
    w2r = np.ascontiguousarray(w2g @ rot)
    b2r = b2 @ rot
    dw1r = np.ascontiguousarray(rot.T @ dw1)

    # even/odd score codebooks [HQ, 2, 128, VOCAB//2]; contraction row 255
    # (k=1, p=127) carries the bias K*(SHIFT - |E|^2)
    e2e = np.zeros((HQ, 2, 128, VOCAB // 2), ml_dtypes.bfloat16)
    e2o = np.zeros((HQ, 2, 128, VOCAB // 2), ml_dtypes.bfloat16)
    e2tp_pack = np.zeros((HQ, 128, VOCAB), np.uint32)
    for lv in range(HQ):
        sc2K = np.float32(2.0 * Ks[lv])
        Et = (sc2K * cbr[lv].T).astype(np.float32)      # [LAT, VOCAB]
        bias = (Ks[lv] * (SHIFTs[lv] - e2sum[lv])).astype(np.float32)
        Et[255, :] = bias                                # stolen dim
        Ebf = Et.astype(ml_dtypes.bfloat16)
        for k in range(2):
            e2e[lv, k] = Ebf[k * 128:(k + 1) * 128, 0::2]
            e2o[lv, k] = Ebf[k * 128:(k + 1) * 128, 1::2]
        # gather table: exact (rotated) codebook, bf16-packed pairs
        Etg = cbr[lv].T.astype(ml_dtypes.bfloat16)       # [LAT, VOCAB]
        pk0 = Etg[:128].view(np.uint16).astype(np.uint32)
        pk1 = Etg[128:].view(np.uint16).astype(np.uint32)
        e2tp_pack[lv] = pk0 | (pk1 << 16)

    common = {
        "w1b": np.ascontiguousarray(w1.astype(ml_dtypes.bfloat16)),
        "b1": np.ascontiguousarray(b1.reshape(HID // 512, 512).astype(ml_dtypes.bfloat16)),
        "w2b": np.ascontiguousarray(w2r.astype(ml_dtypes.bfloat16)),
        "b2": b2r.reshape(1, LAT).astype(ml_dtypes.bfloat16),
        "dw1b": np.ascontiguousarray(dw1r.astype(ml_dtypes.bfloat16)),
        "db1": np.ascontiguousarray(db1.reshape(HID // 512, 512).astype(ml_dtypes.bfloat16)),
        "dw2b": np.ascontiguousarray(dw2.astype(ml_dtypes.bfloat16)),
        "db2": np.ascontiguousarray(db2.reshape(OBS // 512, 512).astype(ml_dtypes.bfloat16)),
        "e2e": np.ascontiguousarray(e2e),
        "e2o": np.ascontiguousarray(e2o),
        "e2tp": e2tp_pack,
        "identb": np.eye(128, dtype=np.float32).astype(ml_dtypes.bfloat16),
        "onesrow": np.ones((1, NSH), ml_dtypes.bfloat16),
        "giota": np.ascontiguousarray(
            np.tile(np.arange(4, dtype=np.float32), (128, RT))
        ),
    }
    flags = dict(
        use_b1=bool(np.any(b1 != 0)),
        use_b2=bool(np.any(b2r != 0)),
        use_db1=bool(np.any(db1 != 0)),
        use_db2=bool(np.any(db2 != 0)),
    )
    in_maps = []
    for c in range(NCORES):
        m = dict(common)
        m["xbt"] = np.ascontiguousarray(
            x[c * NSH:(c + 1) * NSH].T.astype(ml_dtypes.bfloat16)
        )
        in_maps.append(m)
    return in_maps, flags


def _combine(results):
    rlv = rrec = 0.0
    for c in range(NCORES):
        o = np.asarray(results[c]["out"], np.float64)
        rlv += o[:, 0:8].sum()
        rrec += o[:, 8:40].sum()
    return np.float32(1.5 * rlv / (N * LAT) + 0.5 * rrec / (N * OBS))


_NC_CACHE = {}


def get_nc(flags):
    key = tuple(sorted(flags.items()))
    if key not in _NC_CACHE:
        _NC_CACHE[key] = build_nc(**flags)
    return _NC_CACHE[key]


def kernel(**inputs) -> np.ndarray:
    in_maps, flags = _host_prep(inputs)
    nc = get_nc(flags)
    res = run_bass_kernel_spmd(nc, in_maps, core_ids=list(range(NCORES)))
    return _combine(res.results)


# revision 16
# speedup vs baseline: 1.2252x; 1.0524x over previous
# Trainium2 Bass kernel for residual-VQ autoencoder loss (vq_codebook).
# Data-parallel over rows: 8 NeuronCores, 2048 rows each; codebooks/weights
# replicated. The scalar loss is assembled on host from per-core [128,40]
# partial-sum outputs (no on-chip collectives needed).
#
# Per core (RT=16 row-tiles of 128):
#   encoder: h = x@W1+b1 -> LN -> ReLU -> latent = h@W2' + b2 (ln_g folded
#   into W2; a host-side rotation R is folded into W2/dec_w1/codebooks so
#   latent dim 255 carries minimal residual variance).
#   RVQ pair-max argmax:
#     vocab columns are split even/odd; per scan-group of 1024 pairs the PE
#     writes evens to PSUM-E and odds to PSUM-O (bf16 matmuls). The score
#     bias K*(SHIFT-|E|^2) rides contraction row 255 (the sacrificed dim:
#     residTb row 127 of the k=1 tile is constant 1.0).
#     ACT copies PSUM-O -> SBUF f32 (PSUM has one DVE read port); the custom
#     DVE op QPACK2 then computes per element
#       packed = quant2048(max(odd,even)) + 2*i + (odd>=even)
#     and MAX-accumulates => packed argmax over the group; local idx = packed
#     mod 2048 directly equals the within-group vocab index.
#     idx -> q = codebook[idx] via gpsimd.dma_gather (exact rotated bf16);
#     resid -= q^T.
#   loss telescopes: sum_l 1.5*mean((q_l-r_l)^2) = 1.5*(R0-R4)/(N*LAT);
#   quant^T = latent^T - resid^T; decoder stays feature-major;
#   Rrec = sum((recon-x)^2) using x^T spilled to DRAM during the encoder.

import sys

sys.path.insert(0, "/opt/trn_rl_repo")

import numpy as np

import concourse.bass as bass
import concourse.mybir as mybir
import concourse.bacc as bacc
import concourse.tile as tile
from concourse.alu_op_type import AluOpType
from concourse.bass_utils import run_bass_kernel_spmd

OBS, HID, LAT = 1024, 2048, 256
VOCAB, HQ = 8192, 4
N, NCORES = 16384, 8
NSH = N // NCORES          # 2048 rows per core
RT = NSH // 128            # 16 row tiles
LN_EPS = 1e-5
GRID = 2048.0               # 11 bits: 2*i + b per 1024-pair scan group
BIG = float(1.5 * 2.0**34)  # ulp(BIG)=2048 -> (x+BIG)-BIG rounds to mult of GRID

f32 = mybir.dt.float32
f32r = mybir.dt.float32r
bf16 = mybir.dt.bfloat16
i16 = mybir.dt.int16

_QPACK2 = None


def _register_qpack2():
    """packed = quant2048(max(Src0,Src1)) + 2*idx + (Src0>=Src1);
    accum_out = max over free dim.  Src0: odd scores (SBUF f32, ACT-copied
    from PSUM); Src1: even scores (PSUM f32).  Scores are pre-biased in the
    matmul (bias rides contraction row 255).  With Src0=odds, the local
    vocab index within the 2048-wide group is exactly packed mod 2048."""
    global _QPACK2
    if _QPACK2 is not None:
        return _QPACK2
    from concourse import dve_ops
    from concourse.dve_spec import (
        Spec, Src0, Src1, C1, C2, AluOp, Idx, lower, Bin, Zero, Scan,
    )
    from concourse.dve_table_gen import DveOpSpec

    def _ref(in0, in1, s0, s1, imm2):
        o = np.asarray(in0, np.float32)
        e = np.asarray(in1, np.float32)
        m = np.maximum(o, e)
        q = (m + np.float32(imm2)).astype(np.float32) - np.float32(imm2)
        idx2 = np.arange(o.shape[-1], dtype=np.float32) * np.float32(s1)
        out = ((q + idx2) + (o >= e).astype(np.float32)).astype(np.float32)
        acc = out.max(axis=-1, keepdims=True)
        return out, acc

    m = Bin(AluOp.MAX, Src0, Src1)
    q = Bin(AluOp.SUBTRACT, Bin(AluOp.ADD, m, C2), C2)
    idx2 = Scan(AluOp.ADD, C1, init=Bin(AluOp.SUBTRACT, Zero, C1))
    g = Bin(AluOp.IS_GE, Src0, Src1)
    body = Bin(AluOp.ADD, Bin(AluOp.ADD, q, idx2), g)
    spec = Spec(body=body, accum=AluOp.MAX, reference=_ref)
    op = dve_ops.DveOp("QPACK2_PAIRMAX", spec, subdim=False, uops_sha={})
    dve_ops.OPS.append(op)
    dve_ops.CUSTOM_DVE_SPECS[op.name] = op.spec
    dve_ops._SUB_OPCODE_FOR_NAME[op.name] = (
        dve_ops._CUSTOM_DVE_ROW_BASE + len(dve_ops.OPS) - 1
    )
    for ver in ("v3", "v4"):
        s = DveOpSpec(
            name=op.name,
            opcode=dve_ops.get_dve_sub_opcode(op.name),
            uops=lower(spec, ver=ver),
            rd1_en=True,
        )
        op.uops_sha[ver] = s.sha(ver)
    _QPACK2 = op
    return op


def build_nc(use_b1=True, use_b2=True, use_db1=True, use_db2=True):
    qpack2 = _register_qpack2()
    nc = bacc.Bacc(
        "TRN2",
        target_bir_lowering=False,
        debug=False,
        enable_asserts=False,
        num_devices=NCORES,
    )
    Relu = mybir.ActivationFunctionType.Relu
    Square = mybir.ActivationFunctionType.Square
    Sqrt = mybir.ActivationFunctionType.Sqrt
    Copy = mybir.ActivationFunctionType.Copy

    # ---------------- DRAM I/O ----------------
    x_d = nc.dram_tensor("xbt", [OBS, NSH], bf16, kind="ExternalInput")
    w1_d = nc.dram_tensor("w1b", [OBS, HID], bf16, kind="ExternalInput")
    b1_d = nc.dram_tensor("b1", [HID // 512, 512], bf16, kind="ExternalInput")
    w2_d = nc.dram_tensor("w2b", [HID, LAT], bf16, kind="ExternalInput")
    b2_d = nc.dram_tensor("b2", [1, LAT], bf16, kind="ExternalInput")
    dw1_d = nc.dram_tensor("dw1b", [LAT, HID], bf16, kind="ExternalInput")
    db1_d = nc.dram_tensor("db1", [HID // 512, 512], bf16, kind="ExternalInput")
    dw2_d = nc.dram_tensor("dw2b", [HID, OBS], bf16, kind="ExternalInput")
    db2_d = nc.dram_tensor("db2", [OBS // 512, 512], bf16, kind="ExternalInput")
    # even/odd codebooks: [HQ, k, 128, VOCAB//2]; row (k=1,p=127) = bias
    e2e_d = nc.dram_tensor("e2e", [HQ, 2, 128, VOCAB // 2], bf16, kind="ExternalInput")
    e2o_d = nc.dram_tensor("e2o", [HQ, 2, 128, VOCAB // 2], bf16, kind="ExternalInput")
    e2tp_d = nc.dram_tensor("e2tp", [HQ, 128, VOCAB], mybir.dt.uint32, kind="ExternalInput")
    identb_d = nc.dram_tensor("identb", [128, 128], bf16, kind="ExternalInput")
    ones_d = nc.dram_tensor("onesrow", [1, NSH], bf16, kind="ExternalInput")
    d_giota = nc.dram_tensor("giota", [128, RT * 4], f32, kind="ExternalInput")
    out_d = nc.dram_tensor("out", [128, 40], f32, kind="ExternalOutput")

    import contextlib

    with tile.TileContext(nc) as tc, contextlib.ExitStack() as ctx:
        const_p = ctx.enter_context(tc.tile_pool(name="const", bufs=1))
        persist_p = ctx.enter_context(tc.tile_pool(name="persist", bufs=1))
        small_p = ctx.enter_context(tc.tile_pool(name="small", bufs=4))

        # ---- constants ----
        ones1 = const_p.tile([1, 512], bf16, name="ones1")
        nc.vector.memset(ones1[:], 1.0)
        out_sb = const_p.tile([128, 40], f32, name="out_sb")
        nc.vector.memset(out_sb[:], 0.0)
        epsc = const_p.tile([128, 1], f32, name="epsc")
        nc.vector.memset(epsc[:], LN_EPS)

        # ---- persistent ----
        latT = persist_p.tile([128, 2, NSH], f32, name="latT")
        residT = persist_p.tile([128, 2, NSH], f32, name="residT")
        residTb = persist_p.tile([128, 2, NSH], bf16, name="residTb")
        trash = persist_p.tile([128, 2048], bf16, name="trash")
        trash2 = persist_p.tile([128, 2048], bf16, name="trash2")
        dw1s = persist_p.tile([128, 2, HID], bf16, name="dw1s")
        idx16 = persist_p.tile([128, RT], i16, name="idx16")
        idxg = persist_p.tile([128, RT, 8], i16, name="idxg")
        nc.vector.memset(idxg[:], 0)
        # contraction row 255 of the scores is the bias row: resid slot = 1.0
        # (engine ops can't start at partition 127; write it via DMA)
        nc.sync.dma_start(residTb[127:128, 1, :], ones_d.ap())
        # =============== encoder ===============
        enc_ctx = contextlib.ExitStack()
        enc_p = enc_ctx.enter_context(tc.tile_pool(name="encp", bufs=1))
        work_p = enc_ctx.enter_context(tc.tile_pool(name="encw", bufs=2))
        eps_h = enc_ctx.enter_context(tc.tile_pool(name="epsh", bufs=6, space="PSUM"))
        w2s = enc_p.tile([128, HID // 128, LAT], bf16, name="w2s")
        w1s = enc_p.tile([128, OBS // 128, HID], bf16, name="w1s")
        xT_sb = enc_p.tile([128, OBS // 128, NSH], bf16, name="xT_sb")
        # order: x^T first (needed by the first matmul), then w1, then w2
        for k in range(OBS // 128):
            nc.sync.dma_start(xT_sb[:, k, :], x_d.ap()[k * 128:(k + 1) * 128, :])
        for k in range(OBS // 128):
            nc.sync.dma_start(w1s[:, k, :], w1_d.ap()[k * 128:(k + 1) * 128, :])
        for k in range(HID // 128):
            nc.sync.dma_start(w2s[:, k, :], w2_d.ap()[k * 128:(k + 1) * 128, :])
        b1s = const_p.tile([HID // 512, 512], bf16, name="b1s")
        if use_b1:
            nc.sync.dma_start(b1s[:], b1_d.ap())
        b2s = const_p.tile([1, LAT], bf16, name="b2s")
        if use_b2:
            nc.sync.dma_start(b2s[:], b2_d.ap())
        for k in range(2):
            nc.sync.dma_start(dw1s[:, k, :], dw1_d.ap()[k * 128:(k + 1) * 128, :])

        for rc in range(8):  # row chunks of 256 (2 row tiles each)
            # [p, rj, o, r]: rj-major so each transpose dest is contiguous
            hTc = work_p.tile([128, 2, HID // 128, 128], bf16, name="hTc", tag="hT")
            for rj in range(2):
                rt = rc * 2 + rj
                # h = x @ W1 (+ b1), chunk-grained PSUM for overlap
                hsb = work_p.tile([128, 2048], f32, name="hsb", tag="hsb")
                bns = small_p.tile([128, 4, 6], f32, name="bns", tag="s1")
                for cc in range(4):
                    hps = eps_h.tile([128, 512], f32, name="hps", tag="hps")
                    nmm = OBS // 128
                    for k in range(nmm):
                        nc.tensor.matmul(
                            hps[:],
                            xT_sb[:, k, rt * 128:(rt + 1) * 128],
                            w1s[:, k, cc * 512:(cc + 1) * 512],
                            start=(k == 0),
                            stop=(k == nmm - 1 and not use_b1),
                        )
                    if use_b1:
                        nc.tensor.matmul(
                            hps[:],
                            ones1[:, 0:128],
                            b1s[cc:cc + 1, :],
                            start=False, stop=True,
                        )
                    nc.scalar.activation(
                        hsb[:, cc * 512:(cc + 1) * 512], hps[:],
                        mybir.ActivationFunctionType.Copy,
                    )
                    nc.vector.bn_stats(bns[:, cc, :], hps[:])
                mv = small_p.tile([128, 2], f32, name="mv", tag="s2")
                nc.vector.bn_aggr(mv[:], bns[:].rearrange("p a b -> p (a b)"))
                std = small_p.tile([128, 1], f32, name="std", tag="s6")
                nc.scalar.activation(std[:], mv[:, 1:2], Sqrt, bias=epsc[:])
                rstd = small_p.tile([128, 1], f32, name="rstd", tag="s7")
                nc.vector.reciprocal(rstd[:], std[:])
                nmr = small_p.tile([128, 1], f32, name="nmr", tag="s8")
                nc.vector.tensor_scalar(
                    nmr[:], mv[:, 0:1], rstd[:], -1.0,
                    op0=AluOpType.mult, op1=AluOpType.mult,
                )
                hrelu = work_p.tile([128, 2048], bf16, name="hrelu", tag="hrelu")
                nc.scalar.activation(
                    hrelu[:], hsb[:], Relu, bias=nmr[:], scale=rstd[:]
                )
                # h^T for this row tile via the DMA crossbar transpose
                nc.sync.dma_start_transpose(hTc[:, rj], hrelu[:])
            # latent^T for these 256 rows
            for m in range(2):
                lps = eps_h.tile([128, 256], f32, name="lps", tag="hps")
                nk = HID // 128
                for k in range(nk):
                    nc.tensor.matmul(
                        lps[:, 0:256],
                        w2s[:, k, m * 128:(m + 1) * 128],
                        hTc[:, :, k, :],
                        start=(k == 0),
                        stop=(k == nk - 1 and not use_b2),
                    )
                if use_b2:
                    nc.tensor.matmul(
                        lps[:, 0:256],
                        b2s[:, m * 128:(m + 1) * 128],
                        ones1[:, 0:256],
                        start=False, stop=True,
                    )
                nc.vector.tensor_copy(
                    latT[:, m, rc * 256:(rc + 1) * 256], lps[:, 0:256]
                )

        nc.vector.tensor_copy(residT[:, 0, :], latT[:, 0, :])
        nc.vector.tensor_copy(residT[:, 1, :], latT[:, 1, :])
        nc.vector.tensor_copy(residTb[:, 0, :], latT[:, 0, :])
        nc.vector.tensor_copy(residTb[0:127, 1, :], latT[0:127, 1, :])

        enc_ctx.close()

        # =============== RVQ ===============
        vq_ctx = contextlib.ExitStack()
        vq_p = vq_ctx.enter_context(tc.tile_pool(name="vqp", bufs=1))
        vps_p = vq_ctx.enter_context(tc.tile_pool(name="vps", bufs=2, space="PSUM"))
        sod_p = vq_ctx.enter_context(tc.tile_pool(name="sod", bufs=2))
        # [p, buf, k, c]: double-buffered even/odd codebooks
        e2es = vq_p.tile([128, 2, 2, VOCAB // 2], bf16, name="e2es")
        e2os = vq_p.tile([128, 2, 2, VOCAB // 2], bf16, name="e2os")
        e2tp = vq_p.tile([128, 2, VOCAB], mybir.dt.uint32, name="e2tp")
        qp = vq_p.tile([128, NSH], mybir.dt.uint32, name="qp")
        from concourse import library_config
        nc.gpsimd.load_library(library_config.ap_gather)
        pk64 = persist_p.tile([128, RT * 4], f32, name="pk64")
        giota = const_p.tile([128, RT * 4], f32, name="giota")
        nc.sync.dma_start(giota[:], d_giota.ap())

        def load_level(lv):
            db = lv % 2
            for k in range(2):
                nc.sync.dma_start(e2es[:, db, k, :], e2e_d.ap()[lv, k])
                nc.sync.dma_start(e2os[:, db, k, :], e2o_d.ap()[lv, k])
            nc.sync.dma_start(e2tp[:, db, :], e2tp_d.ap()[lv])

        qb = qp[:].bitcast(bf16).rearrange("p (n two) -> p n two", two=2)

        def do_update(lv, uq):
            """Apply quarter uq's quantization: resid -= q. For lv<HQ-1 also
            refresh residTb (scores operand, rows 0..254 + constant row 255);
            for the last level write quant = lat - resid into residTb instead
            (the decoder consumes it there; row writes must start at 0/96)."""
            c0, c1 = uq * 512, (uq + 1) * 512
            for m in range(2):
                nc.vector.tensor_sub(
                    residT[:, m, c0:c1].rearrange("p (n o) -> p n o", o=1),
                    residT[:, m, c0:c1].rearrange("p (n o) -> p n o", o=1),
                    qb[:, c0:c1, m:m + 1],
                )
            if lv < HQ - 1:
                nc.vector.tensor_copy(residTb[:, 0, c0:c1], residT[:, 0, c0:c1])
                nc.vector.tensor_copy(
                    residTb[0:127, 1, c0:c1], residT[0:127, 1, c0:c1]
                )
            else:
                nc.vector.tensor_sub(
                    residTb[:, 0, c0:c1], latT[:, 0, c0:c1], residT[:, 0, c0:c1]
                )
                nc.vector.tensor_sub(
                    residTb[0:96, 1, c0:c1], latT[0:96, 1, c0:c1],
                    residT[0:96, 1, c0:c1],
                )
                nc.vector.tensor_sub(
                    residTb[96:128, 1, c0:c1], latT[96:128, 1, c0:c1],
                    residT[96:128, 1, c0:c1],
                )

        def do_squares(lv):
            for m in range(2):
                nc.scalar.activation(
                    trash2[:], residT[:, m, :], Square,
                    accum_out=out_sb[:, 2 * lv + m: 2 * lv + m + 1],
                )

        load_level(0)
        for lv in range(HQ):
            db = lv % 2
            if lv + 1 < HQ:
                load_level(lv + 1)
            if lv > 0:
                # deferred last-quarter update + loss accumulation of the
                # previous level (keeps the DVE off the gather critical path)
                do_update(lv - 1, 3)
                do_squares(lv - 1)
            for qq in range(4):  # quarters of 4 row tiles, pipelined
                for rj in range(4):
                    rt = qq * 4 + rj
                    for g in range(4):
                        psE = vps_p.tile([128, 1024], f32, name="psE", tag="psE")
                        psO = vps_p.tile([128, 1024], f32, name="psO", tag="psO")
                        sodd = sod_p.tile([128, 1024], f32, name="sodd", tag="sodd")
                        for h in range(2):
                            c0 = g * 1024 + h * 512
                            for k in range(2):
                                nc.tensor.matmul(
                                    psO[:, h * 512:(h + 1) * 512],
                                    residTb[:, k, rt * 128:(rt + 1) * 128],
                                    e2os[:, db, k, c0:c0 + 512],
                                    start=(k == 0), stop=(k == 1),
                                )
                        for h in range(2):
                            c0 = g * 1024 + h * 512
                            for k in range(2):
                                nc.tensor.matmul(
                                    psE[:, h * 512:(h + 1) * 512],
                                    residTb[:, k, rt * 128:(rt + 1) * 128],
                                    e2es[:, db, k, c0:c0 + 512],
                                    start=(k == 0), stop=(k == 1),
                                )
                        nc.scalar.activation(sodd[:], psO[:], Copy)
                        nc.vector._custom_dve(
                            qpack2,
                            out=trash[:, 0:1024],
                            in0=sodd[:],
                            in1=psE[:],
                            s1=2.0,
                            imm2=BIG,
                            accum_out=pk64[:, rt * 4 + g: rt * 4 + g + 1],
                        )
                # index extraction for this quarter
                cs = qq * 16
                pk3 = pk64[:, cs:cs + 16].rearrange("p (a b) -> p a b", a=4)
                m16 = small_p.tile([128, 4], f32, name="m16", tag="m16")
                nc.vector.tensor_reduce(
                    m16[:], pk3, axis=mybir.AxisListType.X, op=AluOpType.max
                )
                msk = small_p.tile([128, 4, 4], f32, name="msk", tag="msk")
                nc.vector.tensor_tensor(
                    msk[:], pk3,
                    m16[:].rearrange("p (a o) -> p a o", o=1)
                    .broadcast_to((128, 4, 4)),
                    op=AluOpType.is_ge,
                )
                nc.vector.tensor_mul(
                    msk[:], msk[:],
                    giota[:, cs:cs + 16].rearrange("p (a b) -> p a b", a=4),
                )
                gidx = small_p.tile([128, 4], f32, name="gidx", tag="gidx")
                nc.vector.tensor_reduce(
                    gidx[:], msk[:], axis=mybir.AxisListType.X, op=AluOpType.add
                )
                nc.vector.tensor_scalar_min(gidx[:], gidx[:], 3.0)
                mi = small_p.tile([128, 4], mybir.dt.int32, name="mi", tag="mi")
                nc.vector.tensor_copy(mi[:], m16[:])
                nc.vector.tensor_scalar(
                    mi[:], mi[:], int(GRID) - 1, None, op0=AluOpType.bitwise_and
                )
                loc = small_p.tile([128, 4], f32, name="loc", tag="loc")
                nc.vector.tensor_copy(loc[:], mi[:])
                nc.vector.tensor_scalar(
                    gidx[:], gidx[:], GRID, None, op0=AluOpType.mult
                )
                nc.vector.tensor_add(loc[:], loc[:], gidx[:])
                nc.vector.tensor_copy(idx16[:, qq * 4:(qq + 1) * 4], loc[:])
                # stage indices (wrapped + replicated across 8 Q7 groups)
                for kk in range(8):
                    nc.gpsimd.dma_start(
                        idxg[0:16, qq * 4:(qq + 1) * 4, kk],
                        idx16[kk * 16:(kk + 1) * 16, qq * 4:(qq + 1) * 4],
                    )
                for gg in range(1, 8):
                    nc.gpsimd.dma_start(
                        idxg[gg * 16:(gg + 1) * 16, qq * 4:(qq + 1) * 4, :],
                        idxg[0:16, qq * 4:(qq + 1) * 4, :],
                    )
                nc.gpsimd.ap_gather(
                    qp[:, qq * 512:(qq + 1) * 512],
                    e2tp[:, db, :],
                    idxg[:, qq * 4:(qq + 1) * 4, :].rearrange("p a b -> p (a b)"),
                    channels=128, num_elems=VOCAB, d=1, num_idxs=512,
                )
                # apply the PREVIOUS quarter's update here so the in-order
                # DVE never waits on this quarter's gather
                if qq > 0:
                    do_update(lv, qq - 1)
        do_update(HQ - 1, 3)
        do_squares(HQ - 1)

        vq_ctx.close()

        # =============== decoder ===============
        dec_ctx = contextlib.ExitStack()
        dec_p = dec_ctx.enter_context(tc.tile_pool(name="decp", bufs=1))
        work_p = dec_ctx.enter_context(tc.tile_pool(name="decw", bufs=2))
        dps_p = dec_ctx.enter_context(tc.tile_pool(name="dps", bufs=4, space="PSUM"))
        dw2s = dec_p.tile([128, HID // 128, OBS], bf16, name="dw2s")
        for k in range(HID // 128):
            nc.sync.dma_start(dw2s[:, k, :], dw2_d.ap()[k * 128:(k + 1) * 128, :])
        db1s = const_p.tile([HID // 512, 512], bf16, name="db1s")
        if use_db1:
            nc.sync.dma_start(db1s[:], db1_d.ap())
        db2s = const_p.tile([OBS // 512, 512], bf16, name="db2s")
        if use_db2:
            nc.sync.dma_start(db2s[:], db2_d.ap())

        for rc in range(4):  # row chunks of 512
            dhT = work_p.tile([128, HID // 128, 512], bf16, name="dhT", tag="hT")
            for ht in range(HID // 128):
                dps = dps_p.tile([128, 512], f32, name="dps", tag="dmm")
                for k in range(2):
                    nc.tensor.matmul(
                        dps[:, 0:512],
                        dw1s[:, k, ht * 128:(ht + 1) * 128],
                        residTb[:, k, rc * 512:(rc + 1) * 512],
                        start=(k == 0), stop=(k == 1 and not use_db1),
                    )
                if use_db1:
                    nc.tensor.matmul(
                        dps[:, 0:512],
                        db1s[(ht * 128) // 512:(ht * 128) // 512 + 1,
                             (ht * 128) % 512:(ht * 128) % 512 + 128],
                        ones1[:],
                        start=False, stop=True,
                    )
                nc.scalar.activation(dhT[:, ht, :], dps[:, 0:512], Relu)
            for ot in range(OBS // 128):
                xTl = work_p.tile([128, 512], bf16, name="xTl", tag="xTl")
                nc.sync.dma_start(
                    xTl[:],
                    x_d.ap()[ot * 128:(ot + 1) * 128, rc * 512:(rc + 1) * 512],
                )
                rps = dps_p.tile([128, 512], f32, name="rps", tag="dmm")
                nk = HID // 128
                for k in range(nk):
                    nc.tensor.matmul(
                        rps[:, 0:512],
                        dw2s[:, k, ot * 128:(ot + 1) * 128],
                        dhT[:, k, :],
                        start=(k == 0), stop=(k == nk - 1 and not use_db2),
                    )
                if use_db2:
                    nc.tensor.matmul(
                        rps[:, 0:512],
                        db2s[(ot * 128) // 512:(ot * 128) // 512 + 1,
                             (ot * 128) % 512:(ot * 128) % 512 + 128],
                        ones1[:],
                        start=False, stop=True,
                    )
                diff = work_p.tile([128, 512], f32, name="diff", tag="diff")
                nc.vector.tensor_sub(diff[:], rps[:, 0:512], xTl[:])
                nc.scalar.activation(
                    diff[:], diff[:], Square,
                    accum_out=out_sb[:, 8 + rc * 8 + ot: 9 + rc * 8 + ot],
                )

        dec_ctx.close()
        nc.sync.dma_start(out_d.ap(), out_sb[:])

    nc.compile()
    return nc


def _host_prep(inputs):
    import ml_dtypes

    x = np.asarray(inputs["x"], np.float32)
    cb = np.ascontiguousarray(np.asarray(inputs["codebooks"], np.float32))
    w1 = np.ascontiguousarray(np.asarray(inputs["enc_w1"], np.float32))
    b1 = np.asarray(inputs["enc_b1"], np.float32)
    lng = np.asarray(inputs["ln_g"], np.float32)
    lnb = np.asarray(inputs["ln_b"], np.float32)
    w2 = np.asarray(inputs["enc_w2"], np.float32)
    b2 = np.asarray(inputs["enc_b2"], np.float32)
    dw1 = np.ascontiguousarray(np.asarray(inputs["dec_w1"], np.float32))
    db1 = np.asarray(inputs["dec_b1"], np.float32)
    dw2 = np.asarray(inputs["dec_w2"], np.float32)
    db2 = np.asarray(inputs["dec_b2"], np.float32)

    assert np.all(lnb == 0.0) and np.all(lng > 0.0), "kernel assumes ln_b==0, ln_g>0"
    w2g = w2 * lng[:, None]  # relu(z*g)@W2 == relu(z)@(g[:,None]*W2) for g>0

    e2sum = (cb.astype(np.float64) ** 2).sum(-1).astype(np.float32)  # [HQ, VOCAB]

    # sample rows: estimate per-level score ranges AND the residual covariance
    # (for the rotation that minimizes variance of the sacrificed dim 255)
    rng = np.random.default_rng(0)
    sel = rng.choice(x.shape[0], 1024, replace=False)
    h = x[sel] @ w1 + b1
    mu = h.mean(-1, keepdims=True)
    var = ((h - mu) ** 2).mean(-1, keepdims=True)
    h = np.maximum((h - mu) / np.sqrt(var + LN_EPS) * lng + lnb, 0.0)
    lat_s = h @ w2 + b2
    resid = lat_s.copy()
    pool = [resid.copy()]
    Ks, SHIFTs = [], []
    for lv in range(HQ):
        sc = 2.0 * resid @ cb[lv].T - e2sum[lv]
        lo, hi = float(sc.min()), float(sc.max())
        span = hi - lo
        shift = -lo + 0.75 * span + 16.0       # margin: scores stay well positive
        smax = (hi + shift) * 2.0              # 2x safety for sample underestimate
        K = np.float32((2.0**24 * 0.98) / smax)
        Ks.append(K)
        SHIFTs.append(np.float32(shift))
        idx = sc.argmax(-1)
        resid = resid - cb[lv][idx]
        if lv < HQ - 1:
            pool.append(resid.copy())
    P = np.concatenate(pool, 0)
    C = np.cov(P.T)
    w_eig, V = np.linalg.eigh(C)               # ascending eigenvalues
    rot = np.ascontiguousarray(V[:, ::-1])     # col 255 = min-variance direction

    cbr = np.ascontiguousarray(cb @ rot)       # rotated codebooks [HQ, V, LAT]
    w2r = np.ascontiguousarray(w2g @ rot)
    b2r = b2 @ rot
    dw1r = np.ascontiguousarray(rot.T @ dw1)

    # even/odd score codebooks [HQ, 2, 128, VOCAB//2]; contraction row 255
    # (k=1, p=127) carries the bias K*(SHIFT - |E|^2)
    e2e = np.zeros((HQ, 2, 128, VOCAB // 2), ml_dtypes.bfloat16)
    e2o = np.zeros((HQ, 2, 128, VOCAB // 2), ml_dtypes.bfloat16)
    e2tp_pack = np.zeros((HQ, 128, VOCAB), np.uint32)
    for lv in range(HQ):
        sc2K = np.float32(2.0 * Ks[lv])
        Et = (sc2K * cbr[lv].T).astype(np.float32)      # [LAT, VOCAB]
        bias = (Ks[lv] * (SHIFTs[lv] - e2sum[lv])).astype(np.float32)
        Et[255, :] = bias                                # stolen dim
        Ebf = Et.astype(ml_dtypes.bfloat16)
        for k in range(2):
            e2e[lv, k] = Ebf[k * 128:(k + 1) * 128, 0::2]
            e2o[lv, k] = Ebf[k * 128:(k + 1) * 128, 1::2]
        # gather table: exact (rotated) codebook, bf16-packed pairs
        Etg = cbr[lv].T.astype(ml_dtypes.bfloat16)       # [LAT, VOCAB]
        pk0 = Etg[:128].view(np.uint16).astype(np.uint32)
        pk1 = Etg[128:].view(np.uint16).astype(np.uint32)
        e2tp_pack[lv] = pk0 | (pk1 << 16)

    common = {
        "w1b": np.ascontiguousarray(w1.astype(ml_dtypes.bfloat16)),
        "b1": np.ascontiguousarray(b1.reshape(HID // 512, 512).astype(ml_dtypes.bfloat16)),
        "w2b": np.ascontiguousarray(w2r.astype(ml_dtypes.bfloat16)),
        "b2": b2r.reshape(1, LAT).astype(ml_dtypes.bfloat16),
        "dw1b": np.ascontiguousarray(dw1r.astype(ml_dtypes.bfloat16)),
        "db1": np.ascontiguousarray(db1.reshape(HID // 512, 512).astype(ml_dtypes.bfloat16)),
        "dw2b": np.ascontiguousarray(dw2.astype(ml_dtypes.bfloat16)),
        "db2": np.ascontiguousarray(db2.reshape(OBS // 512, 512).astype(ml_dtypes.bfloat16)),
        "e2e": np.ascontiguousarray(e2e),
        "e2o": np.ascontiguousarray(e2o),
        "e2tp": e2tp_pack,
        "identb": np.eye(128, dtype=np.float32).astype(ml_dtypes.bfloat16),
        "onesrow": np.ones((1, NSH), ml_dtypes.bfloat16),
        "giota": np.ascontiguousarray(
            np.tile(np.arange(4, dtype=np.float32), (128, RT))
        ),
    }
    flags = dict(
        use_b1=bool(np.any(b1 != 0)),
        use_b2=bool(np.any(b2r != 0)),
        use_db1=bool(np.any(db1 != 0)),
        use_db2=bool(np.any(db2 != 0)),
    )
    in_maps = []
    for c in range(NCORES):
        m = dict(common)
        m["xbt"] = np.ascontiguousarray(
            x[c * NSH:(c + 1) * NSH].T.astype(ml_dtypes.bfloat16)
        )
        in_maps.append(m)
    return in_maps, flags


def _combine(results):
    rlv = rrec = 0.0
    for c in range(NCORES):
        o = np.asarray(results[c]["out"], np.float64)
        rlv += o[:, 0:8].sum()
        rrec += o[:, 8:40].sum()
    return np.float32(1.5 * rlv / (N * LAT) + 0.5 * rrec / (N * OBS))


_NC_CACHE = {}


def get_nc(flags):
    key = tuple(sorted(flags.items()))
    if key not in _NC_CACHE:
        _NC_CACHE[key] = build_nc(**flags)
    return _NC_CACHE[key]


def kernel(**inputs) -> np.ndarray:
    in_maps, flags = _host_prep(inputs)
    nc = get_nc(flags)
    res = run_bass_kernel_spmd(nc, in_maps, core_ids=list(range(NCORES)))
    return _combine(res.results)


# revision 25
# speedup vs baseline: 1.2384x; 1.0107x over previous
# Trainium2 Bass kernel for residual-VQ autoencoder loss (vq_codebook).
# Data-parallel over rows: 8 NeuronCores, 2048 rows each; codebooks/weights
# replicated. The scalar loss is assembled on host from per-core [128,40]
# partial-sum outputs (no on-chip collectives needed).
#
# Per core (RT=16 row-tiles of 128):
#   encoder: h = x@W1+b1 -> LN -> ReLU -> latent = h@W2' + b2 (ln_g folded
#   into W2; a host-side rotation R is folded into W2/dec_w1/codebooks so
#   latent dim 255 carries minimal residual variance).
#   RVQ pair-max argmax:
#     vocab columns are split even/odd; per scan-group of 1024 pairs the PE
#     writes evens to PSUM-E and odds to PSUM-O (bf16 matmuls). The score
#     bias K*(SHIFT-|E|^2) rides contraction row 255 (the sacrificed dim:
#     residTb row 127 of the k=1 tile is constant 1.0).
#     ACT copies PSUM-O -> SBUF f32 (PSUM has one DVE read port); the custom
#     DVE op QPACK2 then computes per element
#       packed = quant2048(max(odd,even)) + 2*i + (odd>=even)
#     and MAX-accumulates => packed argmax over the group; local idx = packed
#     mod 2048 directly equals the within-group vocab index.
#     idx -> q = codebook[idx] via gpsimd.dma_gather (exact rotated bf16);
#     resid -= q^T.
#   loss telescopes: sum_l 1.5*mean((q_l-r_l)^2) = 1.5*(R0-R4)/(N*LAT);
#   quant^T = latent^T - resid^T; decoder stays feature-major;
#   Rrec = sum((recon-x)^2) using x^T spilled to DRAM during the encoder.

import sys

sys.path.insert(0, "/opt/trn_rl_repo")

import numpy as np

import concourse.bass as bass
import concourse.mybir as mybir
import concourse.bacc as bacc
import concourse.tile as tile
from concourse.alu_op_type import AluOpType
from concourse.bass_utils import run_bass_kernel_spmd

OBS, HID, LAT = 1024, 2048, 256
VOCAB, HQ = 8192, 4
N, NCORES = 16384, 8
NSH = N // NCORES          # 2048 rows per core
RT = NSH // 128            # 16 row tiles
LN_EPS = 1e-5
GRID = 2048.0               # 11 bits: 2*i + b per 1024-pair scan group
BIG = float(1.5 * 2.0**34)  # ulp(BIG)=2048 -> (x+BIG)-BIG rounds to mult of GRID

f32 = mybir.dt.float32
f32r = mybir.dt.float32r
bf16 = mybir.dt.bfloat16
i16 = mybir.dt.int16

_QPACK2 = None


def _register_qpack2():
    """packed = quant2048(max(Src0,Src1)) + 2*idx + (Src0>=Src1);
    accum_out = max over free dim.  Src0: odd scores (SBUF f32, ACT-copied
    from PSUM); Src1: even scores (PSUM f32).  Scores are pre-biased in the
    matmul (bias rides contraction row 255).  With Src0=odds, the local
    vocab index within the 2048-wide group is exactly packed mod 2048."""
    global _QPACK2
    if _QPACK2 is not None:
        return _QPACK2
    from concourse import dve_ops
    from concourse.dve_spec import (
        Spec, Src0, Src1, C1, C2, AluOp, Idx, lower, Bin, Zero, Scan,
    )
    from concourse.dve_table_gen import DveOpSpec

    def _ref(in0, in1, s0, s1, imm2):
        o = np.asarray(in0, np.float32)
        e = np.asarray(in1, np.float32)
        m = np.maximum(o, e)
        q = (m + np.float32(imm2)).astype(np.float32) - np.float32(imm2)
        idx2 = np.arange(o.shape[-1], dtype=np.float32) * np.float32(s1)
        out = ((q + idx2) + (o >= e).astype(np.float32)).astype(np.float32)
        acc = out.max(axis=-1, keepdims=True)
        return out, acc

    m = Bin(AluOp.MAX, Src0, Src1)
    q = Bin(AluOp.SUBTRACT, Bin(AluOp.ADD, m, C2), C2)
    idx2 = Scan(AluOp.ADD, C1, init=Bin(AluOp.SUBTRACT, Zero, C1))
    g = Bin(AluOp.IS_GE, Src0, Src1)
    body = Bin(AluOp.ADD, Bin(AluOp.ADD, q, idx2), g)
    spec = Spec(body=body, accum=AluOp.MAX, reference=_ref)
    op = dve_ops.DveOp("QPACK2_PAIRMAX", spec, subdim=False, uops_sha={})
    dve_ops.OPS.append(op)
    dve_ops.CUSTOM_DVE_SPECS[op.name] = op.spec
    dve_ops._SUB_OPCODE_FOR_NAME[op.name] = (
        dve_ops._CUSTOM_DVE_ROW_BASE + len(dve_ops.OPS) - 1
    )
    for ver in ("v3", "v4"):
        s = DveOpSpec(
            name=op.name,
            opcode=dve_ops.get_dve_sub_opcode(op.name),
            uops=lower(spec, ver=ver),
            rd1_en=True,
        )
        op.uops_sha[ver] = s.sha(ver)
    _QPACK2 = op
    return op


def build_nc(use_b1=True, use_b2=True, use_db1=True, use_db2=True):
    qpack2 = _register_qpack2()
    nc = bacc.Bacc(
        "TRN2",
        target_bir_lowering=False,
        debug=False,
        enable_asserts=False,
        num_devices=NCORES,
    )
    Relu = mybir.ActivationFunctionType.Relu
    Square = mybir.ActivationFunctionType.Square
    Sqrt = mybir.ActivationFunctionType.Sqrt
    Copy = mybir.ActivationFunctionType.Copy

    # ---------------- DRAM I/O ----------------
    x_d = nc.dram_tensor("xbt", [OBS, NSH], bf16, kind="ExternalInput")
    w1_d = nc.dram_tensor("w1b", [OBS, HID], bf16, kind="ExternalInput")
    b1_d = nc.dram_tensor("b1", [HID // 512, 512], bf16, kind="ExternalInput")
    w2_d = nc.dram_tensor("w2b", [HID, LAT], bf16, kind="ExternalInput")
    b2_d = nc.dram_tensor("b2", [1, LAT], bf16, kind="ExternalInput")
    dw1_d = nc.dram_tensor("dw1b", [LAT, HID], bf16, kind="ExternalInput")
    db1_d = nc.dram_tensor("db1", [HID // 512, 512], bf16, kind="ExternalInput")
    dw2_d = nc.dram_tensor("dw2b", [HID, OBS], bf16, kind="ExternalInput")
    db2_d = nc.dram_tensor("db2", [OBS // 512, 512], bf16, kind="ExternalInput")
    # even/odd codebooks: [HQ, k, 128, VOCAB//2]; row (k=1,p=127) = bias
    e2e_d = nc.dram_tensor("e2e", [HQ, 2, 128, VOCAB // 2], bf16, kind="ExternalInput")
    e2o_d = nc.dram_tensor("e2o", [HQ, 2, 128, VOCAB // 2], bf16, kind="ExternalInput")
    e2tp_d = nc.dram_tensor("e2tp", [HQ, 128, VOCAB], mybir.dt.uint32, kind="ExternalInput")
    ones_d = nc.dram_tensor("onesrow", [1, NSH], bf16, kind="ExternalInput")
    d_giota = nc.dram_tensor("giota", [128, RT * 4], f32, kind="ExternalInput")
    out_d = nc.dram_tensor("out", [128, 64], f32, kind="ExternalOutput")

    import contextlib

    with tile.TileContext(nc) as tc, contextlib.ExitStack() as ctx:
        const_p = ctx.enter_context(tc.tile_pool(name="const", bufs=1))
        persist_p = ctx.enter_context(tc.tile_pool(name="persist", bufs=1))
        small_p = ctx.enter_context(tc.tile_pool(name="small", bufs=4))

        # ---- constants ----
        ones1 = const_p.tile([1, 512], bf16, name="ones1")
        nc.vector.memset(ones1[:], 1.0)
        out_sb = const_p.tile([128, 64], f32, name="out_sb")
        nc.vector.memset(out_sb[:], 0.0)
        epsc = const_p.tile([128, 1], f32, name="epsc")
        nc.vector.memset(epsc[:], LN_EPS)

        # ---- persistent ----
        latT = persist_p.tile([128, 2, NSH], f32, name="latT")
        residT = persist_p.tile([128, 2, NSH], f32, name="residT")
        residTb = persist_p.tile([128, 2, NSH], bf16, name="residTb")
        trash = persist_p.tile([128, 2048], bf16, name="trash")
        trash2 = persist_p.tile([128, 2048], bf16, name="trash2")
        dw1s = persist_p.tile([128, 2, HID], bf16, name="dw1s")
        idx16 = persist_p.tile([128, RT], i16, name="idx16")
        idxg = persist_p.tile([128, RT, 8], i16, name="idxg")
        nc.vector.memset(idxg[:], 0)
        # contraction row 255 of the scores is the bias row: resid slot = 1.0
        # (engine ops can't start at partition 127; write it via DMA)
        nc.sync.dma_start(residTb[127:128, 1, :], ones_d.ap())
        # =============== encoder ===============
        enc_ctx = contextlib.ExitStack()
        enc_p = enc_ctx.enter_context(tc.tile_pool(name="encp", bufs=1))
        work_p = enc_ctx.enter_context(tc.tile_pool(name="encw", bufs=3))
        eps_h = enc_ctx.enter_context(tc.tile_pool(name="epsh", bufs=6, space="PSUM"))
        w2s = enc_p.tile([128, HID // 128, LAT], bf16, name="w2s")
        w1s = enc_p.tile([128, OBS // 128, HID], bf16, name="w1s")
        xT_sb = enc_p.tile([128, OBS // 128, NSH], bf16, name="xT_sb")
        # order: x^T first (needed by the first matmul), then w1, then w2
        for k in range(OBS // 128):
            nc.sync.dma_start(xT_sb[:, k, :], x_d.ap()[k * 128:(k + 1) * 128, :])
        # w1 by output-column chunk so the first h-matmul group starts sooner
        for cc in range(4):
            for k in range(OBS // 128):
                nc.sync.dma_start(
                    w1s[:, k, cc * 512:(cc + 1) * 512],
                    w1_d.ap()[k * 128:(k + 1) * 128, cc * 512:(cc + 1) * 512],
                )
        for k in range(HID // 128):
            nc.sync.dma_start(w2s[:, k, :], w2_d.ap()[k * 128:(k + 1) * 128, :])
        b1s = const_p.tile([HID // 512, 512], bf16, name="b1s")
        if use_b1:
            nc.sync.dma_start(b1s[:], b1_d.ap())
        b2s = const_p.tile([1, LAT], bf16, name="b2s")
        if use_b2:
            nc.sync.dma_start(b2s[:], b2_d.ap())
        for k in range(2):
            nc.sync.dma_start(dw1s[:, k, :], dw1_d.ap()[k * 128:(k + 1) * 128, :])

        for rc in range(8):  # row chunks of 256 (2 row tiles each)
            # [p, rj, o, r]: rj-major so each transpose dest is contiguous
            hTc = work_p.tile([128, 2, HID // 128, 128], bf16, name="hTc", tag="hT")
            for rj in range(2):
                rt = rc * 2 + rj
                # h = x @ W1 (+ b1), chunk-grained PSUM for overlap
                hsb = work_p.tile([128, 2048], f32, name="hsb", tag="hsb")
                bns = small_p.tile([128, 4, 6], f32, name="bns", tag="s1")
                for cc in range(4):
                    hps = eps_h.tile([128, 512], f32, name="hps", tag="hps")
                    nmm = OBS // 128
                    for k in range(nmm):
                        nc.tensor.matmul(
                            hps[:],
                            xT_sb[:, k, rt * 128:(rt + 1) * 128],
                            w1s[:, k, cc * 512:(cc + 1) * 512],
                            start=(k == 0),
                            stop=(k == nmm - 1 and not use_b1),
                        )
                    if use_b1:
                        nc.tensor.matmul(
                            hps[:],
                            ones1[:, 0:128],
                            b1s[cc:cc + 1, :],
                            start=False, stop=True,
                        )
                    nc.scalar.activation(
                        hsb[:, cc * 512:(cc + 1) * 512], hps[:],
                        mybir.ActivationFunctionType.Copy,
                    )
                    nc.vector.bn_stats(bns[:, cc, :], hps[:])
                mv = small_p.tile([128, 2], f32, name="mv", tag="s2")
                nc.vector.bn_aggr(mv[:], bns[:].rearrange("p a b -> p (a b)"))
                std = small_p.tile([128, 1], f32, name="std", tag="s6")
                nc.scalar.activation(std[:], mv[:, 1:2], Sqrt, bias=epsc[:])
                rstd = small_p.tile([128, 1], f32, name="rstd", tag="s7")
                nc.vector.reciprocal(rstd[:], std[:])
                nmr = small_p.tile([128, 1], f32, name="nmr", tag="s8")
                nc.vector.tensor_scalar(
                    nmr[:], mv[:, 0:1], rstd[:], -1.0,
                    op0=AluOpType.mult, op1=AluOpType.mult,
                )
                hrelu = work_p.tile([128, 2048], bf16, name="hrelu", tag="hrelu")
                nc.scalar.activation(
                    hrelu[:], hsb[:], Relu, bias=nmr[:], scale=rstd[:]
                )
                # h^T for this row tile via the DMA crossbar transpose
                nc.sync.dma_start_transpose(hTc[:, rj], hrelu[:])
            # latent^T for these 256 rows
            for m in range(2):
                lps = eps_h.tile([128, 256], f32, name="lps", tag="hps")
                nk = HID // 128
                for k in range(nk):
                    nc.tensor.matmul(
                        lps[:, 0:256],
                        w2s[:, k, m * 128:(m + 1) * 128],
                        hTc[:, :, k, :],
                        start=(k == 0),
                        stop=(k == nk - 1 and not use_b2),
                    )
                if use_b2:
                    nc.tensor.matmul(
                        lps[:, 0:256],
                        b2s[:, m * 128:(m + 1) * 128],
                        ones1[:, 0:256],
                        start=False, stop=True,
                    )
                nc.vector.tensor_copy(
                    latT[:, m, rc * 256:(rc + 1) * 256], lps[:, 0:256]
                )

        nc.vector.tensor_copy(residT[:, 0, :], latT[:, 0, :])
        nc.vector.tensor_copy(residT[:, 1, :], latT[:, 1, :])
        nc.vector.tensor_copy(residTb[:, 0, :], latT[:, 0, :])
        nc.vector.tensor_copy(residTb[0:127, 1, :], latT[0:127, 1, :])

        enc_ctx.close()

        # =============== RVQ ===============
        vq_ctx = contextlib.ExitStack()
        vq_p = vq_ctx.enter_context(tc.tile_pool(name="vqp", bufs=1))
        vps_p = vq_ctx.enter_context(tc.tile_pool(name="vps", bufs=2, space="PSUM"))
        sod_p = vq_ctx.enter_context(tc.tile_pool(name="sod", bufs=2))
        # [p, buf, k, c]: double-buffered even/odd codebooks
        e2es = vq_p.tile([128, 2, 2, VOCAB // 2], bf16, name="e2es")
        e2os = vq_p.tile([128, 2, 2, VOCAB // 2], bf16, name="e2os")
        e2tp = vq_p.tile([128, 2, VOCAB], mybir.dt.uint32, name="e2tp")
        qp = vq_p.tile([128, NSH], mybir.dt.uint32, name="qp")
        from concourse import library_config
        nc.gpsimd.load_library(library_config.ap_gather)
        pk64 = persist_p.tile([128, RT * 4], f32, name="pk64")
        giota = const_p.tile([128, RT * 4], f32, name="giota")
        nc.sync.dma_start(giota[:], d_giota.ap())

        def load_level(lv):
            db = lv % 2
            for k in range(2):
                nc.sync.dma_start(e2es[:, db, k, :], e2e_d.ap()[lv, k])
                nc.sync.dma_start(e2os[:, db, k, :], e2o_d.ap()[lv, k])
            nc.sync.dma_start(e2tp[:, db, :], e2tp_d.ap()[lv])

        qb = qp[:].bitcast(bf16).rearrange("p (n two) -> p n two", two=2)

        def extraction_ops(lv, qq):
            """Index extraction + gather staging for quarter (lv, qq), as a
            list of closures (issued interleaved between QPACK2 groups)."""
            cs = qq * 16
            db = lv % 2
            ops = []
            m16 = small_p.tile([128, 4], f32, name="m16", tag="m16")
            msk = small_p.tile([128, 4, 4], f32, name="msk", tag="msk")
            gidx = small_p.tile([128, 4], f32, name="gidx", tag="gidx")
            mi = small_p.tile([128, 4], mybir.dt.int32, name="mi", tag="mi")
            loc = small_p.tile([128, 4], f32, name="loc", tag="loc")
            pk3 = pk64[:, cs:cs + 16].rearrange("p (a b) -> p a b", a=4)

            def s1():
                nc.vector.tensor_reduce(
                    m16[:], pk3, axis=mybir.AxisListType.X, op=AluOpType.max
                )
                nc.vector.tensor_tensor(
                    msk[:], pk3,
                    m16[:].rearrange("p (a o) -> p a o", o=1)
                    .broadcast_to((128, 4, 4)),
                    op=AluOpType.is_ge,
                )
                nc.vector.tensor_mul(
                    msk[:], msk[:],
                    giota[:, cs:cs + 16].rearrange("p (a b) -> p a b", a=4),
                )

            def s2():
                nc.vector.tensor_reduce(
                    gidx[:], msk[:], axis=mybir.AxisListType.X, op=AluOpType.add
                )
                nc.vector.tensor_scalar_min(gidx[:], gidx[:], 3.0)
                nc.vector.tensor_copy(mi[:], m16[:])
                nc.vector.tensor_scalar(
                    mi[:], mi[:], int(GRID) - 1, None, op0=AluOpType.bitwise_and
                )

            def s3():
                nc.vector.tensor_copy(loc[:], mi[:])
                nc.vector.tensor_scalar(
                    gidx[:], gidx[:], GRID, None, op0=AluOpType.mult
                )
                nc.vector.tensor_add(loc[:], loc[:], gidx[:])
                nc.vector.tensor_copy(idx16[:, qq * 4:(qq + 1) * 4], loc[:])

            def s4():
                # stage indices (wrapped + replicated across 8 Q7 groups)
                for kk in range(8):
                    nc.gpsimd.dma_start(
                        idxg[0:16, qq * 4:(qq + 1) * 4, kk],
                        idx16[kk * 16:(kk + 1) * 16, qq * 4:(qq + 1) * 4],
                    )
                for gg in range(1, 8):
                    nc.gpsimd.dma_start(
                        idxg[gg * 16:(gg + 1) * 16, qq * 4:(qq + 1) * 4, :],
                        idxg[0:16, qq * 4:(qq + 1) * 4, :],
                    )
                nc.gpsimd.ap_gather(
                    qp[:, qq * 512:(qq + 1) * 512],
                    e2tp[:, db, :],
                    idxg[:, qq * 4:(qq + 1) * 4, :].rearrange("p a b -> p (a b)"),
                    channels=128, num_elems=VOCAB, d=1, num_idxs=512,
                )

            return [s1, s2, s3, s4]

        def update_ops(lv, qq):
            """resid -= q for quarter (lv, qq) + its loss square. For lv<HQ-1
            also refresh residTb; for the last level write quant = lat - resid
            into residTb (decoder input; engine writes start at 0/96 only)."""
            c0, c1 = qq * 512, (qq + 1) * 512

            def u(m):
                def f():
                    nc.vector.tensor_sub(
                        residT[:, m, c0:c1].rearrange("p (n o) -> p n o", o=1),
                        residT[:, m, c0:c1].rearrange("p (n o) -> p n o", o=1),
                        qb[:, c0:c1, m:m + 1],
                    )
                    if lv < HQ - 1:
                        if m == 0:
                            nc.vector.tensor_copy(
                                residTb[:, 0, c0:c1], residT[:, 0, c0:c1]
                            )
                        else:
                            nc.vector.tensor_copy(
                                residTb[0:127, 1, c0:c1], residT[0:127, 1, c0:c1]
                            )
                    else:
                        if m == 0:
                            nc.vector.tensor_sub(
                                residTb[:, 0, c0:c1],
                                latT[:, 0, c0:c1], residT[:, 0, c0:c1],
                            )
                        else:
                            nc.vector.tensor_sub(
                                residTb[0:96, 1, c0:c1],
                                latT[0:96, 1, c0:c1], residT[0:96, 1, c0:c1],
                            )
                            nc.vector.tensor_sub(
                                residTb[96:128, 1, c0:c1],
                                latT[96:128, 1, c0:c1], residT[96:128, 1, c0:c1],
                            )
                    # per-quarter loss accumulation (col lv*8 + m*4 + qq)
                    nc.scalar.activation(
                        trash2[:, 0:512], residT[:, m, c0:c1], Square,
                        accum_out=out_sb[:, lv * 8 + m * 4 + qq:
                                         lv * 8 + m * 4 + qq + 1],
                    )
                return f

            return [u(0), u(1)]

        def quarter_groups(lv, qq, deferred):
            """Issue the 16 matmul/QPACK2 groups of quarter (lv, qq), with the
            deferred closures of older quarters spread between row tiles."""
            db = lv % 2
            chunks = [deferred[i::4] for i in range(4)]
            for rj in range(4):
                rt = qq * 4 + rj
                for g in range(4):
                    psE = vps_p.tile([128, 1024], f32, name="psE", tag="psE")
                    psO = vps_p.tile([128, 1024], f32, name="psO", tag="psO")
                    sodd = sod_p.tile([128, 1024], f32, name="sodd", tag="sodd")
                    for h in range(2):
                        c0 = g * 1024 + h * 512
                        for k in range(2):
                            nc.tensor.matmul(
                                psO[:, h * 512:(h + 1) * 512],
                                residTb[:, k, rt * 128:(rt + 1) * 128],
                                e2os[:, db, k, c0:c0 + 512],
                                start=(k == 0), stop=(k == 1),
                            )
                    for h in range(2):
                        c0 = g * 1024 + h * 512
                        for k in range(2):
                            nc.tensor.matmul(
                                psE[:, h * 512:(h + 1) * 512],
                                residTb[:, k, rt * 128:(rt + 1) * 128],
                                e2es[:, db, k, c0:c0 + 512],
                                start=(k == 0), stop=(k == 1),
                            )
                    nc.scalar.activation(sodd[:], psO[:], Copy)
                    nc.vector._custom_dve(
                        qpack2,
                        out=trash[:, 0:1024],
                        in0=sodd[:],
                        in1=psE[:],
                        s1=2.0,
                        imm2=BIG,
                        accum_out=pk64[:, rt * 4 + g: rt * 4 + g + 1],
                    )
                for f in chunks[rj]:
                    f()

        load_level(0)
        NQ = HQ * 4
        for Q in range(NQ + 2):
            lv, qq = divmod(Q, 4)
            # prefetch at qq==1: the deferred gather of (lv-1, q3) — issued
            # during qq==0 — must read the old e2tp buffer first
            if Q < NQ and qq == 1 and lv + 1 < HQ:
                load_level(lv + 1)
            deferred = []
            if 1 <= Q <= NQ:
                l1, q1 = divmod(Q - 1, 4)
                deferred += extraction_ops(l1, q1)
            if Q >= 2:
                l2, q2 = divmod(Q - 2, 4)
                deferred += update_ops(l2, q2)
            if Q < NQ:
                quarter_groups(lv, qq, deferred)
            else:
                for f in deferred:
                    f()

        vq_ctx.close()

        # =============== decoder ===============
        dec_ctx = contextlib.ExitStack()
        dec_p = dec_ctx.enter_context(tc.tile_pool(name="decp", bufs=1))
        work_p = dec_ctx.enter_context(tc.tile_pool(name="decw", bufs=2))
        dps_p = dec_ctx.enter_context(tc.tile_pool(name="dps", bufs=4, space="PSUM"))
        dw2s = dec_p.tile([128, HID // 128, OBS], bf16, name="dw2s")
        for k in range(HID // 128):
            nc.sync.dma_start(dw2s[:, k, :], dw2_d.ap()[k * 128:(k + 1) * 128, :])
        db1s = const_p.tile([HID // 512, 512], bf16, name="db1s")
        if use_db1:
            nc.sync.dma_start(db1s[:], db1_d.ap())
        db2s = const_p.tile([OBS // 512, 512], bf16, name="db2s")
        if use_db2:
            nc.sync.dma_start(db2s[:], db2_d.ap())

        for rc in range(4):  # row chunks of 512
            dhT = work_p.tile([128, HID // 128, 512], bf16, name="dhT", tag="hT")
            for ht in range(HID // 128):
                dps = dps_p.tile([128, 512], f32, name="dps", tag="dmm")
                for k in range(2):
                    nc.tensor.matmul(
                        dps[:, 0:512],
                        dw1s[:, k, ht * 128:(ht + 1) * 128],
                        residTb[:, k, rc * 512:(rc + 1) * 512],
                        start=(k == 0), stop=(k == 1 and not use_db1),
                    )
                if use_db1:
                    nc.tensor.matmul(
                        dps[:, 0:512],
                        db1s[(ht * 128) // 512:(ht * 128) // 512 + 1,
                             (ht * 128) % 512:(ht * 128) % 512 + 128],
                        ones1[:],
                        start=False, stop=True,
                    )
                nc.scalar.activation(dhT[:, ht, :], dps[:, 0:512], Relu)
            for ot in range(OBS // 128):
                xTl = work_p.tile([128, 512], bf16, name="xTl", tag="xTl")
                nc.sync.dma_start(
                    xTl[:],
                    x_d.ap()[ot * 128:(ot + 1) * 128, rc * 512:(rc + 1) * 512],
                )
                rps = dps_p.tile([128, 512], f32, name="rps", tag="dmm")
                nk = HID // 128
                for k in range(nk):
                    nc.tensor.matmul(
                        rps[:, 0:512],
                        dw2s[:, k, ot * 128:(ot + 1) * 128],
                        dhT[:, k, :],
                        start=(k == 0), stop=(k == nk - 1 and not use_db2),
                    )
                if use_db2:
                    nc.tensor.matmul(
                        rps[:, 0:512],
                        db2s[(ot * 128) // 512:(ot * 128) // 512 + 1,
                             (ot * 128) % 512:(ot * 128) % 512 + 128],
                        ones1[:],
                        start=False, stop=True,
                    )
                diff = work_p.tile([128, 512], f32, name="diff", tag="diff")
                nc.vector.tensor_sub(diff[:], rps[:, 0:512], xTl[:])
                nc.scalar.activation(
                    diff[:], diff[:], Square,
                    accum_out=out_sb[:, 32 + rc * 8 + ot: 33 + rc * 8 + ot],
                )

        dec_ctx.close()
        nc.sync.dma_start(out_d.ap(), out_sb[:])

    nc.compile()
    return nc


def _host_prep(inputs):
    import ml_dtypes

    x = np.asarray(inputs["x"], np.float32)
    cb = np.ascontiguousarray(np.asarray(inputs["codebooks"], np.float32))
    w1 = np.ascontiguousarray(np.asarray(inputs["enc_w1"], np.float32))
    b1 = np.asarray(inputs["enc_b1"], np.float32)
    lng = np.asarray(inputs["ln_g"], np.float32)
    lnb = np.asarray(inputs["ln_b"], np.float32)
    w2 = np.asarray(inputs["enc_w2"], np.float32)
    b2 = np.asarray(inputs["enc_b2"], np.float32)
    dw1 = np.ascontiguousarray(np.asarray(inputs["dec_w1"], np.float32))
    db1 = np.asarray(inputs["dec_b1"], np.float32)
    dw2 = np.asarray(inputs["dec_w2"], np.float32)
    db2 = np.asarray(inputs["dec_b2"], np.float32)

    assert np.all(lnb == 0.0) and np.all(lng > 0.0), "kernel assumes ln_b==0, ln_g>0"
    w2g = w2 * lng[:, None]  # relu(z*g)@W2 == relu(z)@(g[:,None]*W2) for g>0

    e2sum = (cb.astype(np.float64) ** 2).sum(-1).astype(np.float32)  # [HQ, VOCAB]

    # sample rows: estimate per-level score ranges AND the residual covariance
    # (for the rotation that minimizes variance of the sacrificed dim 255)
    rng = np.random.default_rng(0)
    sel = rng.choice(x.shape[0], 1024, replace=False)
    h = x[sel] @ w1 + b1
    mu = h.mean(-1, keepdims=True)
    var = ((h - mu) ** 2).mean(-1, keepdims=True)
    h = np.maximum((h - mu) / np.sqrt(var + LN_EPS) * lng + lnb, 0.0)
    lat_s = h @ w2 + b2
    resid = lat_s.copy()
    pool = [resid.copy()]
    Ks, SHIFTs = [], []
    for lv in range(HQ):
        sc = 2.0 * resid @ cb[lv].T - e2sum[lv]
        lo, hi = float(sc.min()), float(sc.max())
        span = hi - lo
        shift = -lo + 0.75 * span + 16.0       # margin: scores stay well positive
        smax = (hi + shift) * 2.0              # 2x safety for sample underestimate
        K = np.float32((2.0**24 * 0.98) / smax)
        Ks.append(K)
        SHIFTs.append(np.float32(shift))
        idx = sc.argmax(-1)
        resid = resid - cb[lv][idx]
        if lv < HQ - 1:
            pool.append(resid.copy())
    P = np.concatenate(pool, 0)
    C = np.cov(P.T)
    w_eig, V = np.linalg.eigh(C)               # ascending eigenvalues
    rot = np.ascontiguousarray(V[:, ::-1])     # col 255 = min-variance direction

    cbr = np.ascontiguousarray(cb @ rot)       # rotated codebooks [HQ, V, LAT]
    w2r = np.ascontiguousarray(w2g @ rot)
    b2r = b2 @ rot
    dw1r = np.ascontiguousarray(rot.T @ dw1)

    # even/odd score codebooks [HQ, 2, 128, VOCAB//2]; contraction row 255
    # (k=1, p=127) carries the bias K*(SHIFT - |E|^2)
    e2e = np.zeros((HQ, 2, 128, VOCAB // 2), ml_dtypes.bfloat16)
    e2o = np.zeros((HQ, 2, 128, VOCAB // 2), ml_dtypes.bfloat16)
    e2tp_pack = np.zeros((HQ, 128, VOCAB), np.uint32)
    for lv in range(HQ):
        sc2K = np.float32(2.0 * Ks[lv])
        Et = (sc2K * cbr[lv].T).astype(np.float32)      # [LAT, VOCAB]
        bias = (Ks[lv] * (SHIFTs[lv] - e2sum[lv])).astype(np.float32)
        Et[255, :] = bias                                # stolen dim
        Ebf = Et.astype(ml_dtypes.bfloat16)
        for k in range(2):
            e2e[lv, k] = Ebf[k * 128:(k + 1) * 128, 0::2]
            e2o[lv, k] = Ebf[k * 128:(k + 1) * 128, 1::2]
        # gather table: exact (rotated) codebook, bf16-packed pairs
        Etg = cbr[lv].T.astype(ml_dtypes.bfloat16)       # [LAT, VOCAB]
        pk0 = Etg[:128].view(np.uint16).astype(np.uint32)
        pk1 = Etg[128:].view(np.uint16).astype(np.uint32)
        e2tp_pack[lv] = pk0 | (pk1 << 16)

    common = {
        "w1b": np.ascontiguousarray(w1.astype(ml_dtypes.bfloat16)),
        "b1": np.ascontiguousarray(b1.reshape(HID // 512, 512).astype(ml_dtypes.bfloat16)),
        "w2b": np.ascontiguousarray(w2r.astype(ml_dtypes.bfloat16)),
        "b2": b2r.reshape(1, LAT).astype(ml_dtypes.bfloat16),
        "dw1b": np.ascontiguousarray(dw1r.astype(ml_dtypes.bfloat16)),
        "db1": np.ascontiguousarray(db1.reshape(HID // 512, 512).astype(ml_dtypes.bfloat16)),
        "dw2b": np.ascontiguousarray(dw2.astype(ml_dtypes.bfloat16)),
        "db2": np.ascontiguousarray(db2.reshape(OBS // 512, 512).astype(ml_dtypes.bfloat16)),
        "e2e": np.ascontiguousarray(e2e),
        "e2o": np.ascontiguousarray(e2o),
        "e2tp": e2tp_pack,
        "onesrow": np.ones((1, NSH), ml_dtypes.bfloat16),
        "giota": np.ascontiguousarray(
            np.tile(np.arange(4, dtype=np.float32), (128, RT))
        ),
    }
    flags = dict(
        use_b1=bool(np.any(b1 != 0)),
        use_b2=bool(np.any(b2r != 0)),
        use_db1=bool(np.any(db1 != 0)),
        use_db2=bool(np.any(db2 != 0)),
    )
    in_maps = []
    for c in range(NCORES):
        m = dict(common)
        m["xbt"] = np.ascontiguousarray(
            x[c * NSH:(c + 1) * NSH].T.astype(ml_dtypes.bfloat16)
        )
        in_maps.append(m)
    return in_maps, flags


def _combine(results):
    rlv = rrec = 0.0
    for c in range(NCORES):
        o = np.asarray(results[c]["out"], np.float64)
        rlv += o[:, 0:32].sum()
        rrec += o[:, 32:64].sum()
    return np.float32(1.5 * rlv / (N * LAT) + 0.5 * rrec / (N * OBS))


_NC_CACHE = {}


def get_nc(flags):
    key = tuple(sorted(flags.items()))
    if key not in _NC_CACHE:
        _NC_CACHE[key] = build_nc(**flags)
    return _NC_CACHE[key]


def kernel(**inputs) -> np.ndarray:
    in_maps, flags = _host_prep(inputs)
    nc = get_nc(flags)
    res = run_bass_kernel_spmd(nc, in_maps, core_ids=list(range(NCORES)))
    return _combine(res.results)


# revision 29
# speedup vs baseline: 1.3419x; 1.0836x over previous
# Trainium2 Bass kernel for residual-VQ autoencoder loss (vq_codebook).
# Data-parallel over rows: 8 NeuronCores, 2048 rows each; codebooks/weights
# replicated. The scalar loss is assembled on host from per-core [128,40]
# partial-sum outputs (no on-chip collectives needed).
#
# Per core (RT=16 row-tiles of 128):
#   encoder: h = x@W1+b1 -> LN -> ReLU -> latent = h@W2' + b2 (ln_g folded
#   into W2; a host-side rotation R is folded into W2/dec_w1/codebooks so
#   latent dim 255 carries minimal residual variance).
#   RVQ pair-max argmax:
#     vocab columns are split even/odd; per scan-group of 1024 pairs the PE
#     writes evens to PSUM-E and odds to PSUM-O (bf16 matmuls). The score
#     bias K*(SHIFT-|E|^2) rides contraction row 255 (the sacrificed dim:
#     residTb row 127 of the k=1 tile is constant 1.0).
#     ACT copies PSUM-O -> SBUF f32 (PSUM has one DVE read port); the custom
#     DVE op QPACK2 then computes per element
#       packed = quant2048(max(odd,even)) + 2*i + (odd>=even)
#     and MAX-accumulates => packed argmax over the group; local idx = packed
#     mod 2048 directly equals the within-group vocab index.
#     idx -> q = codebook[idx] via gpsimd.dma_gather (exact rotated bf16);
#     resid -= q^T.
#   loss telescopes: sum_l 1.5*mean((q_l-r_l)^2) = 1.5*(R0-R4)/(N*LAT);
#   quant^T = latent^T - resid^T; decoder stays feature-major;
#   Rrec = sum((recon-x)^2) using x^T spilled to DRAM during the encoder.

import sys

sys.path.insert(0, "/opt/trn_rl_repo")

import numpy as np

import concourse.bass as bass
import concourse.mybir as mybir
import concourse.bacc as bacc
import concourse.tile as tile
from concourse.alu_op_type import AluOpType
from concourse.bass_utils import run_bass_kernel_spmd

OBS, HID, LAT = 1024, 2048, 256
VOCAB, HQ = 8192, 4
N, NCORES = 16384, 8
NSH = N // NCORES          # 2048 rows per core
RT = NSH // 128            # 16 row tiles
LN_EPS = 1e-5
GRID = 2048.0               # 11 bits: 2*i + b per 1024-pair scan group
BIG = float(1.5 * 2.0**34)  # ulp(BIG)=2048 -> (x+BIG)-BIG rounds to mult of GRID

f32 = mybir.dt.float32
f32r = mybir.dt.float32r
bf16 = mybir.dt.bfloat16
i16 = mybir.dt.int16

_QPACK2 = None


def _register_qpack2():
    """packed = quant2048(max(Src0,Src1)) + 2*idx + (Src0>=Src1);
    accum_out = max over free dim.  Src0: odd scores (SBUF f32, ACT-copied
    from PSUM); Src1: even scores (PSUM f32).  Scores are pre-biased in the
    matmul (bias rides contraction row 255).  With Src0=odds, the local
    vocab index within the 2048-wide group is exactly packed mod 2048."""
    global _QPACK2
    if _QPACK2 is not None:
        return _QPACK2
    from concourse import dve_ops
    from concourse.dve_spec import (
        Spec, Src0, Src1, C1, C2, AluOp, Idx, lower, Bin, Zero, Scan,
    )
    from concourse.dve_table_gen import DveOpSpec

    def _ref(in0, in1, s0, s1, imm2):
        o = np.asarray(in0, np.float32)
        e = np.asarray(in1, np.float32)
        m = np.maximum(o, e)
        q = (m + np.float32(imm2)).astype(np.float32) - np.float32(imm2)
        idx2 = np.arange(o.shape[-1], dtype=np.float32) * np.float32(s1)
        out = ((q + idx2) + (o >= e).astype(np.float32)).astype(np.float32)
        acc = out.max(axis=-1, keepdims=True)
        return out, acc

    m = Bin(AluOp.MAX, Src0, Src1)
    q = Bin(AluOp.SUBTRACT, Bin(AluOp.ADD, m, C2), C2)
    idx2 = Scan(AluOp.ADD, C1, init=Bin(AluOp.SUBTRACT, Zero, C1))
    g = Bin(AluOp.IS_GE, Src0, Src1)
    body = Bin(AluOp.ADD, Bin(AluOp.ADD, q, idx2), g)
    spec = Spec(body=body, accum=AluOp.MAX, reference=_ref)
    op = dve_ops.DveOp("QPACK2_PAIRMAX", spec, subdim=False, uops_sha={})
    dve_ops.OPS.append(op)
    dve_ops.CUSTOM_DVE_SPECS[op.name] = op.spec
    dve_ops._SUB_OPCODE_FOR_NAME[op.name] = (
        dve_ops._CUSTOM_DVE_ROW_BASE + len(dve_ops.OPS) - 1
    )
    for ver in ("v3", "v4"):
        s = DveOpSpec(
            name=op.name,
            opcode=dve_ops.get_dve_sub_opcode(op.name),
            uops=lower(spec, ver=ver),
            rd1_en=True,
        )
        op.uops_sha[ver] = s.sha(ver)
    _QPACK2 = op
    return op


def build_nc(use_b1=True, use_b2=True, use_db1=True, use_db2=True):
    qpack2 = _register_qpack2()
    nc = bacc.Bacc(
        "TRN2",
        target_bir_lowering=False,
        debug=False,
        enable_asserts=False,
        num_devices=NCORES,
    )
    Relu = mybir.ActivationFunctionType.Relu
    Square = mybir.ActivationFunctionType.Square
    Sqrt = mybir.ActivationFunctionType.Sqrt
    Copy = mybir.ActivationFunctionType.Copy

    # ---------------- DRAM I/O ----------------
    x_d = nc.dram_tensor("xbt", [OBS, NSH], bf16, kind="ExternalInput")
    w1_d = nc.dram_tensor("w1b", [OBS, HID], bf16, kind="ExternalInput")
    b1_d = nc.dram_tensor("b1", [HID // 512, 512], bf16, kind="ExternalInput")
    w2_d = nc.dram_tensor("w2b", [HID, LAT], bf16, kind="ExternalInput")
    b2_d = nc.dram_tensor("b2", [1, LAT], bf16, kind="ExternalInput")
    dw1_d = nc.dram_tensor("dw1b", [LAT, HID], bf16, kind="ExternalInput")
    db1_d = nc.dram_tensor("db1", [HID // 512, 512], bf16, kind="ExternalInput")
    dw2_d = nc.dram_tensor("dw2b", [HID, OBS], bf16, kind="ExternalInput")
    db2_d = nc.dram_tensor("db2", [OBS // 512, 512], bf16, kind="ExternalInput")
    # even/odd codebooks: [HQ, k, 128, VOCAB//2]; row (k=1,p=127) = bias
    e2e_d = nc.dram_tensor("e2e", [HQ, 2, 128, VOCAB // 2], bf16, kind="ExternalInput")
    e2o_d = nc.dram_tensor("e2o", [HQ, 2, 128, VOCAB // 2], bf16, kind="ExternalInput")
    e2tp_d = nc.dram_tensor("e2tp", [HQ, 128, VOCAB], mybir.dt.uint32, kind="ExternalInput")
    ones_d = nc.dram_tensor("onesrow", [1, NSH], bf16, kind="ExternalInput")
    d_giota = nc.dram_tensor("giota", [128, RT * 4], f32, kind="ExternalInput")
    out_d = nc.dram_tensor("out", [128, 64], f32, kind="ExternalOutput")

    import contextlib

    with tile.TileContext(nc) as tc, contextlib.ExitStack() as ctx:
        const_p = ctx.enter_context(tc.tile_pool(name="const", bufs=1))
        persist_p = ctx.enter_context(tc.tile_pool(name="persist", bufs=1))
        small_p = ctx.enter_context(tc.tile_pool(name="small", bufs=4))

        # ---- constants ----
        ones1 = const_p.tile([1, 512], bf16, name="ones1")
        nc.vector.memset(ones1[:], 1.0)
        out_sb = const_p.tile([128, 64], f32, name="out_sb")
        nc.vector.memset(out_sb[:], 0.0)
        epsc = const_p.tile([128, 1], f32, name="epsc")
        nc.vector.memset(epsc[:], LN_EPS)

        # ---- persistent ----
        latT = persist_p.tile([128, 2, NSH], f32, name="latT")
        residT = persist_p.tile([128, 2, NSH], f32, name="residT")
        residTb = persist_p.tile([128, 2, NSH], bf16, name="residTb")
        trash = persist_p.tile([128, 2048], bf16, name="trash")
        trash2 = persist_p.tile([128, 2048], bf16, name="trash2")
        dw1s = persist_p.tile([128, 2, HID], bf16, name="dw1s")
        idx16 = persist_p.tile([128, RT], i16, name="idx16")
        idxg = persist_p.tile([128, RT, 8], i16, name="idxg")
        nc.vector.memset(idxg[:], 0)
        # contraction row 255 of the scores is the bias row: resid slot = 1.0
        # (engine ops can't start at partition 127; write it via DMA)
        nc.sync.dma_start(residTb[127:128, 1, :], ones_d.ap())
        # =============== encoder ===============
        enc_ctx = contextlib.ExitStack()
        enc_p = enc_ctx.enter_context(tc.tile_pool(name="encp", bufs=1))
        work_p = enc_ctx.enter_context(tc.tile_pool(name="encw", bufs=3))
        eps_h = enc_ctx.enter_context(tc.tile_pool(name="epsh", bufs=6, space="PSUM"))
        w2s = enc_p.tile([128, HID // 128, LAT], bf16, name="w2s")
        w1s = enc_p.tile([128, OBS // 128, HID], bf16, name="w1s")
        xT_sb = enc_p.tile([128, OBS // 128, NSH], bf16, name="xT_sb")
        # order: x^T first (needed by the first matmul), then w1, then w2
        for k in range(OBS // 128):
            nc.sync.dma_start(xT_sb[:, k, :], x_d.ap()[k * 128:(k + 1) * 128, :])
        # w1 by output-column chunk so the first h-matmul group starts sooner
        for cc in range(4):
            for k in range(OBS // 128):
                nc.sync.dma_start(
                    w1s[:, k, cc * 512:(cc + 1) * 512],
                    w1_d.ap()[k * 128:(k + 1) * 128, cc * 512:(cc + 1) * 512],
                )
        for k in range(HID // 128):
            nc.sync.dma_start(w2s[:, k, :], w2_d.ap()[k * 128:(k + 1) * 128, :])
        b1s = const_p.tile([HID // 512, 512], bf16, name="b1s")
        if use_b1:
            nc.sync.dma_start(b1s[:], b1_d.ap())
        b2s = const_p.tile([1, LAT], bf16, name="b2s")
        if use_b2:
            nc.sync.dma_start(b2s[:], b2_d.ap())
        for k in range(2):
            nc.sync.dma_start(dw1s[:, k, :], dw1_d.ap()[k * 128:(k + 1) * 128, :])

        def enc_lat(rc, hTc):
            # latent^T for chunk rc's 256 rows (issued one chunk late so the
            # PE never head-of-line blocks on the h^T DMA transposes)
            for m in range(2):
                lps = eps_h.tile([128, 256], f32, name="lps", tag="hps")
                nk = HID // 128
                for k in range(nk):
                    nc.tensor.matmul(
                        lps[:, 0:256],
                        w2s[:, k, m * 128:(m + 1) * 128],
                        hTc[:, :, k, :],
                        start=(k == 0),
                        stop=(k == nk - 1 and not use_b2),
                    )
                if use_b2:
                    nc.tensor.matmul(
                        lps[:, 0:256],
                        b2s[:, m * 128:(m + 1) * 128],
                        ones1[:, 0:256],
                        start=False, stop=True,
                    )
                nc.vector.tensor_copy(
                    latT[:, m, rc * 256:(rc + 1) * 256], lps[:, 0:256]
                )

        prev = None  # (rc, hTc) of the previous chunk
        for rc in range(8):  # row chunks of 256 (2 row tiles each)
            # [p, rj, o, r]: rj-major so each transpose dest is contiguous
            hTc = work_p.tile([128, 2, HID // 128, 128], bf16, name="hTc", tag="hT")
            for rj in range(2):
                rt = rc * 2 + rj
                # h = x @ W1 (+ b1), chunk-grained PSUM for overlap
                hsb = work_p.tile([128, 2048], f32, name="hsb", tag="hsb")
                bns = small_p.tile([128, 4, 6], f32, name="bns", tag="s1")
                for cc in range(4):
                    hps = eps_h.tile([128, 512], f32, name="hps", tag="hps")
                    nmm = OBS // 128
                    for k in range(nmm):
                        nc.tensor.matmul(
                            hps[:],
                            xT_sb[:, k, rt * 128:(rt + 1) * 128],
                            w1s[:, k, cc * 512:(cc + 1) * 512],
                            start=(k == 0),
                            stop=(k == nmm - 1 and not use_b1),
                        )
                    if use_b1:
                        nc.tensor.matmul(
                            hps[:],
                            ones1[:, 0:128],
                            b1s[cc:cc + 1, :],
                            start=False, stop=True,
                        )
                    nc.scalar.activation(
                        hsb[:, cc * 512:(cc + 1) * 512], hps[:],
                        mybir.ActivationFunctionType.Copy,
                    )
                    nc.vector.bn_stats(bns[:, cc, :], hps[:])
                mv = small_p.tile([128, 2], f32, name="mv", tag="s2")
                nc.vector.bn_aggr(mv[:], bns[:].rearrange("p a b -> p (a b)"))
                std = small_p.tile([128, 1], f32, name="std", tag="s6")
                nc.scalar.activation(std[:], mv[:, 1:2], Sqrt, bias=epsc[:])
                rstd = small_p.tile([128, 1], f32, name="rstd", tag="s7")
                nc.vector.reciprocal(rstd[:], std[:])
                nmr = small_p.tile([128, 1], f32, name="nmr", tag="s8")
                nc.vector.tensor_scalar(
                    nmr[:], mv[:, 0:1], rstd[:], -1.0,
                    op0=AluOpType.mult, op1=AluOpType.mult,
                )
                hrelu = work_p.tile([128, 2048], bf16, name="hrelu", tag="hrelu")
                nc.scalar.activation(
                    hrelu[:], hsb[:], Relu, bias=nmr[:], scale=rstd[:]
                )
                # h^T for this row tile via the DMA crossbar transpose
                nc.sync.dma_start_transpose(hTc[:, rj], hrelu[:])
            if prev is not None:
                enc_lat(*prev)
            prev = (rc, hTc)
        enc_lat(*prev)

        nc.vector.tensor_copy(residT[:, 0, :], latT[:, 0, :])
        nc.vector.tensor_copy(residT[:, 1, :], latT[:, 1, :])
        nc.vector.tensor_copy(residTb[:, 0, :], latT[:, 0, :])
        nc.vector.tensor_copy(residTb[0:127, 1, :], latT[0:127, 1, :])

        enc_ctx.close()

        # =============== RVQ ===============
        vq_ctx = contextlib.ExitStack()
        vq_p = vq_ctx.enter_context(tc.tile_pool(name="vqp", bufs=1))
        vps_p = vq_ctx.enter_context(tc.tile_pool(name="vps", bufs=2, space="PSUM"))
        sod_p = vq_ctx.enter_context(tc.tile_pool(name="sod", bufs=2))
        # [p, buf, k, c]: double-buffered even/odd codebooks
        e2es = vq_p.tile([128, 2, 2, VOCAB // 2], bf16, name="e2es")
        e2os = vq_p.tile([128, 2, 2, VOCAB // 2], bf16, name="e2os")
        e2tp = vq_p.tile([128, 2, VOCAB], mybir.dt.uint32, name="e2tp")
        qp = vq_p.tile([128, NSH], mybir.dt.uint32, name="qp")
        from concourse import library_config
        nc.gpsimd.load_library(library_config.ap_gather)
        pk64 = persist_p.tile([128, RT * 4], f32, name="pk64")
        giota = const_p.tile([128, RT * 4], f32, name="giota")
        nc.sync.dma_start(giota[:], d_giota.ap())

        def load_level(lv):
            db = lv % 2
            for k in range(2):
                nc.sync.dma_start(e2es[:, db, k, :], e2e_d.ap()[lv, k])
                nc.sync.dma_start(e2os[:, db, k, :], e2o_d.ap()[lv, k])
            nc.sync.dma_start(e2tp[:, db, :], e2tp_d.ap()[lv])

        qb = qp[:].bitcast(bf16).rearrange("p (n two) -> p n two", two=2)

        def extraction_ops(lv, qq):
            """Index extraction + gather staging for quarter (lv, qq), as a
            list of closures (issued interleaved between QPACK2 groups)."""
            cs = qq * 16
            db = lv % 2
            ops = []
            m16 = small_p.tile([128, 4], f32, name="m16", tag="m16")
            msk = small_p.tile([128, 4, 4], f32, name="msk", tag="msk")
            gidx = small_p.tile([128, 4], f32, name="gidx", tag="gidx")
            mi = small_p.tile([128, 4], mybir.dt.int32, name="mi", tag="mi")
            loc = small_p.tile([128, 4], f32, name="loc", tag="loc")
            pk3 = pk64[:, cs:cs + 16].rearrange("p (a b) -> p a b", a=4)

            def s1():
                nc.vector.tensor_reduce(
                    m16[:], pk3, axis=mybir.AxisListType.X, op=AluOpType.max
                )
                nc.vector.tensor_tensor(
                    msk[:], pk3,
                    m16[:].rearrange("p (a o) -> p a o", o=1)
                    .broadcast_to((128, 4, 4)),
                    op=AluOpType.is_ge,
                )
                nc.vector.tensor_mul(
                    msk[:], msk[:],
                    giota[:, cs:cs + 16].rearrange("p (a b) -> p a b", a=4),
                )

            def s2():
                nc.vector.tensor_reduce(
                    gidx[:], msk[:], axis=mybir.AxisListType.X, op=AluOpType.add
                )
                nc.vector.tensor_scalar_min(gidx[:], gidx[:], 3.0)
                nc.vector.tensor_copy(mi[:], m16[:])
                nc.vector.tensor_scalar(
                    mi[:], mi[:], int(GRID) - 1, None, op0=AluOpType.bitwise_and
                )

            def s3():
                nc.vector.tensor_copy(loc[:], mi[:])
                nc.vector.tensor_scalar(
                    gidx[:], gidx[:], GRID, None, op0=AluOpType.mult
                )
                nc.vector.tensor_add(loc[:], loc[:], gidx[:])
                nc.vector.tensor_copy(idx16[:, qq * 4:(qq + 1) * 4], loc[:])

            def s4():
                # stage indices (wrapped + replicated across 8 Q7 groups)
                for kk in range(8):
                    nc.gpsimd.dma_start(
                        idxg[0:16, qq * 4:(qq + 1) * 4, kk],
                        idx16[kk * 16:(kk + 1) * 16, qq * 4:(qq + 1) * 4],
                    )
                for gg in range(1, 8):
                    nc.gpsimd.dma_start(
                        idxg[gg * 16:(gg + 1) * 16, qq * 4:(qq + 1) * 4, :],
                        idxg[0:16, qq * 4:(qq + 1) * 4, :],
                    )
                nc.gpsimd.ap_gather(
                    qp[:, qq * 512:(qq + 1) * 512],
                    e2tp[:, db, :],
                    idxg[:, qq * 4:(qq + 1) * 4, :].rearrange("p a b -> p (a b)"),
                    channels=128, num_elems=VOCAB, d=1, num_idxs=512,
                )

            return [s1, s2, s3, s4]

        def update_ops(lv, qq):
            """resid -= q for quarter (lv, qq) + its loss square. For lv<HQ-1
            also refresh residTb; for the last level write quant = lat - resid
            into residTb (decoder input; engine writes start at 0/96 only)."""
            c0, c1 = qq * 512, (qq + 1) * 512

            def u(m):
                def f():
                    nc.vector.tensor_sub(
                        residT[:, m, c0:c1].rearrange("p (n o) -> p n o", o=1),
                        residT[:, m, c0:c1].rearrange("p (n o) -> p n o", o=1),
                        qb[:, c0:c1, m:m + 1],
                    )
                    if lv < HQ - 1:
                        if m == 0:
                            nc.vector.tensor_copy(
                                residTb[:, 0, c0:c1], residT[:, 0, c0:c1]
                            )
                        else:
                            nc.vector.tensor_copy(
                                residTb[0:127, 1, c0:c1], residT[0:127, 1, c0:c1]
                            )
                    else:
                        if m == 0:
                            nc.vector.tensor_sub(
                                residTb[:, 0, c0:c1],
                                latT[:, 0, c0:c1], residT[:, 0, c0:c1],
                            )
                        else:
                            nc.vector.tensor_sub(
                                residTb[0:96, 1, c0:c1],
                                latT[0:96, 1, c0:c1], residT[0:96, 1, c0:c1],
                            )
                            nc.vector.tensor_sub(
                                residTb[96:128, 1, c0:c1],
                                latT[96:128, 1, c0:c1], residT[96:128, 1, c0:c1],
                            )
                    # per-quarter loss accumulation (col lv*8 + m*4 + qq)
                    nc.scalar.activation(
                        trash2[:, 0:512], residT[:, m, c0:c1], Square,
                        accum_out=out_sb[:, lv * 8 + m * 4 + qq:
                                         lv * 8 + m * 4 + qq + 1],
                    )
                return f

            return [u(0), u(1)]

        def quarter_groups(lv, qq, chunks):
            """Issue the 16 matmul/QPACK2 groups of quarter (lv, qq), with the
            deferred closures of older quarters spread between row tiles."""
            db = lv % 2
            for rj in range(4):
                rt = qq * 4 + rj
                for g in range(4):
                    psE = vps_p.tile([128, 1024], f32, name="psE", tag="psE")
                    psO = vps_p.tile([128, 1024], f32, name="psO", tag="psO")
                    sodd = sod_p.tile([128, 1024], f32, name="sodd", tag="sodd")
                    for h in range(2):
                        c0 = g * 1024 + h * 512
                        for k in range(2):
                            nc.tensor.matmul(
                                psO[:, h * 512:(h + 1) * 512],
                                residTb[:, k, rt * 128:(rt + 1) * 128],
                                e2os[:, db, k, c0:c0 + 512],
                                start=(k == 0), stop=(k == 1),
                            )
                    for h in range(2):
                        c0 = g * 1024 + h * 512
                        for k in range(2):
                            nc.tensor.matmul(
                                psE[:, h * 512:(h + 1) * 512],
                                residTb[:, k, rt * 128:(rt + 1) * 128],
                                e2es[:, db, k, c0:c0 + 512],
                                start=(k == 0), stop=(k == 1),
                            )
                    nc.scalar.activation(sodd[:], psO[:], Copy)
                    nc.vector._custom_dve(
                        qpack2,
                        out=trash[:, 0:1024],
                        in0=sodd[:],
                        in1=psE[:],
                        s1=2.0,
                        imm2=BIG,
                        accum_out=pk64[:, rt * 4 + g: rt * 4 + g + 1],
                    )
                for f in chunks[rj]:
                    f()

        load_level(0)
        NQ = HQ * 4
        for Q in range(NQ + 2):
            lv, qq = divmod(Q, 4)
            # prefetch at qq==1: the deferred gather of (lv-1, q3) — issued
            # during qq==0 — must read the old e2tp buffer first
            if Q < NQ and qq == 1 and lv + 1 < HQ:
                load_level(lv + 1)
            # chunk schedule: extraction of Q-1 right away (rj0), its gather
            # staged at rj1 (finishes with ~3/4-quarter slack), updates of
            # Q-2 at rj2/rj3 (their gather completed a full quarter ago)
            chunks = [[], [], [], []]
            if 1 <= Q <= NQ:
                l1, q1 = divmod(Q - 1, 4)
                s1, s2, s3, s4 = extraction_ops(l1, q1)
                chunks[0] += [s1, s2, s3]
                chunks[1] += [s4]
            if Q >= 2:
                l2, q2 = divmod(Q - 2, 4)
                u0, u1 = update_ops(l2, q2)
                chunks[2] += [u0]
                chunks[3] += [u1]
            if Q < NQ:
                quarter_groups(lv, qq, chunks)
            else:
                for c in chunks:
                    for f in c:
                        f()

        vq_ctx.close()

        # =============== decoder ===============
        dec_ctx = contextlib.ExitStack()
        dec_p = dec_ctx.enter_context(tc.tile_pool(name="decp", bufs=1))
        work_p = dec_ctx.enter_context(tc.tile_pool(name="decw", bufs=2))
        dps_p = dec_ctx.enter_context(tc.tile_pool(name="dps", bufs=4, space="PSUM"))
        dw2s = dec_p.tile([128, HID // 128, OBS], bf16, name="dw2s")
        for k in range(HID // 128):
            nc.sync.dma_start(dw2s[:, k, :], dw2_d.ap()[k * 128:(k + 1) * 128, :])
        db1s = const_p.tile([HID // 512, 512], bf16, name="db1s")
        if use_db1:
            nc.sync.dma_start(db1s[:], db1_d.ap())
        db2s = const_p.tile([OBS // 512, 512], bf16, name="db2s")
        if use_db2:
            nc.sync.dma_start(db2s[:], db2_d.ap())

        for rc in range(4):  # row chunks of 512
            dhT = work_p.tile([128, HID // 128, 512], bf16, name="dhT", tag="hT")
            for ht in range(HID // 128):
                dps = dps_p.tile([128, 512], f32, name="dps", tag="dmm")
                for k in range(2):
                    nc.tensor.matmul(
                        dps[:, 0:512],
                        dw1s[:, k, ht * 128:(ht + 1) * 128],
                        residTb[:, k, rc * 512:(rc + 1) * 512],
                        start=(k == 0), stop=(k == 1 and not use_db1),
                    )
                if use_db1:
                    nc.tensor.matmul(
                        dps[:, 0:512],
                        db1s[(ht * 128) // 512:(ht * 128) // 512 + 1,
                             (ht * 128) % 512:(ht * 128) % 512 + 128],
                        ones1[:],
                        start=False, stop=True,
                    )
                nc.scalar.activation(dhT[:, ht, :], dps[:, 0:512], Relu)
            for ot in range(OBS // 128):
                xTl = work_p.tile([128, 512], bf16, name="xTl", tag="xTl")
                nc.sync.dma_start(
                    xTl[:],
                    x_d.ap()[ot * 128:(ot + 1) * 128, rc * 512:(rc + 1) * 512],
                )
                rps = dps_p.tile([128, 512], f32, name="rps", tag="dmm")
                nk = HID // 128
                for k in range(nk):
                    nc.tensor.matmul(
                        rps[:, 0:512],
                        dw2s[:, k, ot * 128:(ot + 1) * 128],
                        dhT[:, k, :],
                        start=(k == 0), stop=(k == nk - 1 and not use_db2),
                    )
                if use_db2:
                    nc.tensor.matmul(
                        rps[:, 0:512],
                        db2s[(ot * 128) // 512:(ot * 128) // 512 + 1,
                             (ot * 128) % 512:(ot * 128) % 512 + 128],
                        ones1[:],
                        start=False, stop=True,
                    )
                diff = work_p.tile([128, 512], f32, name="diff", tag="diff")
                nc.vector.tensor_sub(diff[:], rps[:, 0:512], xTl[:])
                nc.scalar.activation(
                    diff[:], diff[:], Square,
                    accum_out=out_sb[:, 32 + rc * 8 + ot: 33 + rc * 8 + ot],
                )

        dec_ctx.close()
        nc.sync.dma_start(out_d.ap(), out_sb[:])

    nc.compile()
    return nc


def _host_prep(inputs):
    import ml_dtypes

    x = np.asarray(inputs["x"], np.float32)
    cb = np.ascontiguousarray(np.asarray(inputs["codebooks"], np.float32))
    w1 = np.ascontiguousarray(np.asarray(inputs["enc_w1"], np.float32))
    b1 = np.asarray(inputs["enc_b1"], np.float32)
    lng = np.asarray(inputs["ln_g"], np.float32)
    lnb = np.asarray(inputs["ln_b"], np.float32)
    w2 = np.asarray(inputs["enc_w2"], np.float32)
    b2 = np.asarray(inputs["enc_b2"], np.float32)
    dw1 = np.ascontiguousarray(np.asarray(inputs["dec_w1"], np.float32))
    db1 = np.asarray(inputs["dec_b1"], np.float32)
    dw2 = np.asarray(inputs["dec_w2"], np.float32)
    db2 = np.asarray(inputs["dec_b2"], np.float32)

    assert np.all(lnb == 0.0) and np.all(lng > 0.0), "kernel assumes ln_b==0, ln_g>0"
    w2g = w2 * lng[:, None]  # relu(z*g)@W2 == relu(z)@(g[:,None]*W2) for g>0

    e2sum = (cb.astype(np.float64) ** 2).sum(-1).astype(np.float32)  # [HQ, VOCAB]

    # sample rows: estimate per-level score ranges AND the residual covariance
    # (for the rotation that minimizes variance of the sacrificed dim 255)
    rng = np.random.default_rng(0)
    sel = rng.choice(x.shape[0], 1024, replace=False)
    h = x[sel] @ w1 + b1
    mu = h.mean(-1, keepdims=True)
    var = ((h - mu) ** 2).mean(-1, keepdims=True)
    h = np.maximum((h - mu) / np.sqrt(var + LN_EPS) * lng + lnb, 0.0)
    lat_s = h @ w2 + b2
    resid = lat_s.copy()
    pool = [resid.copy()]
    Ks, SHIFTs = [], []
    for lv in range(HQ):
        sc = 2.0 * resid @ cb[lv].T - e2sum[lv]
        lo, hi = float(sc.min()), float(sc.max())
        span = hi - lo
        shift = -lo + 0.75 * span + 16.0       # margin: scores stay well positive
        smax = (hi + shift) * 2.0              # 2x safety for sample underestimate
        K = np.float32((2.0**24 * 0.98) / smax)
        Ks.append(K)
        SHIFTs.append(np.float32(shift))
        idx = sc.argmax(-1)
        resid = resid - cb[lv][idx]
        if lv < HQ - 1:
            pool.append(resid.copy())
    P = np.concatenate(pool, 0)
    C = np.cov(P.T)
    w_eig, V = np.linalg.eigh(C)               # ascending eigenvalues
    rot = np.ascontiguousarray(V[:, ::-1])     # col 255 = min-variance direction

    cbr = np.ascontiguousarray(cb @ rot)       # rotated codebooks [HQ, V, LAT]
    w2r = np.ascontiguousarray(w2g @ rot)
    b2r = b2 @ rot
    dw1r = np.ascontiguousarray(rot.T @ dw1)

    # even/odd score codebooks [HQ, 2, 128, VOCAB//2]; contraction row 255
    # (k=1, p=127) carries the bias K*(SHIFT - |E|^2)
    e2e = np.zeros((HQ, 2, 128, VOCAB // 2), ml_dtypes.bfloat16)
    e2o = np.zeros((HQ, 2, 128, VOCAB // 2), ml_dtypes.bfloat16)
    e2tp_pack = np.zeros((HQ, 128, VOCAB), np.uint32)
    for lv in range(HQ):
        sc2K = np.float32(2.0 * Ks[lv])
        Et = (sc2K * cbr[lv].T).astype(np.float32)      # [LAT, VOCAB]
        bias = (Ks[lv] * (SHIFTs[lv] - e2sum[lv])).astype(np.float32)
        Et[255, :] = bias                                # stolen dim
        Ebf = Et.astype(ml_dtypes.bfloat16)
        for k in range(2):
            e2e[lv, k] = Ebf[k * 128:(k + 1) * 128, 0::2]
            e2o[lv, k] = Ebf[k * 128:(k + 1) * 128, 1::2]
        # gather table: exact (rotated) codebook, bf16-packed pairs
        Etg = cbr[lv].T.astype(ml_dtypes.bfloat16)       # [LAT, VOCAB]
        pk0 = Etg[:128].view(np.uint16).astype(np.uint32)
        pk1 = Etg[128:].view(np.uint16).astype(np.uint32)
        e2tp_pack[lv] = pk0 | (pk1 << 16)

    common = {
        "w1b": np.ascontiguousarray(w1.astype(ml_dtypes.bfloat16)),
        "b1": np.ascontiguousarray(b1.reshape(HID // 512, 512).astype(ml_dtypes.bfloat16)),
        "w2b": np.ascontiguousarray(w2r.astype(ml_dtypes.bfloat16)),
        "b2": b2r.reshape(1, LAT).astype(ml_dtypes.bfloat16),
        "dw1b": np.ascontiguousarray(dw1r.astype(ml_dtypes.bfloat16)),
        "db1": np.ascontiguousarray(db1.reshape(HID // 512, 512).astype(ml_dtypes.bfloat16)),
        "dw2b": np.ascontiguousarray(dw2.astype(ml_dtypes.bfloat16)),
        "db2": np.ascontiguousarray(db2.reshape(OBS // 512, 512).astype(ml_dtypes.bfloat16)),
        "e2e": np.ascontiguousarray(e2e),
        "e2o": np.ascontiguousarray(e2o),
        "e2tp": e2tp_pack,
        "onesrow": np.ones((1, NSH), ml_dtypes.bfloat16),
        "giota": np.ascontiguousarray(
            np.tile(np.arange(4, dtype=np.float32), (128, RT))
        ),
    }
    flags = dict(
        use_b1=bool(np.any(b1 != 0)),
        use_b2=bool(np.any(b2r != 0)),
        use_db1=bool(np.any(db1 != 0)),
        use_db2=bool(np.any(db2 != 0)),
    )
    in_maps = []
    for c in range(NCORES):
        m = dict(common)
        m["xbt"] = np.ascontiguousarray(
            x[c * NSH:(c + 1) * NSH].T.astype(ml_dtypes.bfloat16)
        )
        in_maps.append(m)
    return in_maps, flags


def _combine(results):
    rlv = rrec = 0.0
    for c in range(NCORES):
        o = np.asarray(results[c]["out"], np.float64)
        rlv += o[:, 0:32].sum()
        rrec += o[:, 32:64].sum()
    return np.float32(1.5 * rlv / (N * LAT) + 0.5 * rrec / (N * OBS))


_NC_CACHE = {}


def get_nc(flags):
    key = tuple(sorted(flags.items()))
    if key not in _NC_CACHE:
        _NC_CACHE[key] = build_nc(**flags)
    return _NC_CACHE[key]


def kernel(**inputs) -> np.ndarray:
    in_maps, flags = _host_prep(inputs)
    nc = get_nc(flags)
    res = run_bass_kernel_spmd(nc, in_maps, core_ids=list(range(NCORES)))
    return _combine(res.results)
